# revision 11
# baseline (speedup 1.0000x reference)
"""nn_CrossAttention Trainium2 kernel — 8-core data-parallel over batch.

Per core (batch slice b=1):
  Warmup matmul burst un-throttles the PE clock (HAM) while the first
  input slabs stream in.  Both 1x1 convs run interleaved in bf16
  (stationary = 8-row input slabs); h-major PSUM tiles are staged
  contiguously (eviction halves split across DVE/ACT, 3+5 PSUM banks
  deep) then repacked to channel-major SBUF regions by DVE/ACT/gpsimd.
  Depthwise 3x3/7x7 as per-(channel, dh) banded-Toeplitz matmuls on the
  TensorEngine (q/k tables fp8e4m3, v tables fp8e3m4 with per-channel
  power-of-2 scaling descaled via a per-partition multiply on the fused
  projection matrix).  Sum-of-squares for the l2 norms lag behind the
  k-dw loop (q squares on gpsimd, k squares on ACT, reduces on DVE).
  QK^T matmuls for both head-pairs are sprinkled through the v-dw
  matmul stream (their strided moving operands are slow, so they ride
  the dense stream instead of running alone and cold); softmax chains
  and half the pair-0 v^T transposes also interleave with v-dw.  The
  output tail fuses the remaining transposes with the (attn@v)+proj
  matmuls, with one warm-keeper matmul per block so the clock gate
  stays open.
"""

import sys

sys.path.insert(0, "/opt/trn_rl_repo")

import numpy as np
import ml_dtypes

B, C, Himg, Wimg = 8, 192, 128, 128
HW = Himg * Wimg
HEADS, DHC = 4, 48      # heads, channels per head
PC = 96                 # channels per head-pair
SLAB = 8                # image rows per input stream slab
NB0 = 16                # pair-0 v^T blocks transposed during v-dw

_PROG = None            # cached (nc, meta)


def _build_toeplitz(wdw, ksz, dtype, scales=None):
    """wdw [c, ksz, ksz] f32 -> [128, c*ksz, 128], tile index = c*ksz + dh.

    T[w_in, tile, w_out] = wdw[c, dh, w_in - w_out + pad] inside the band,
    else 0.  Partition-major so a wave DMA reads contiguous bytes per
    partition.  Optional per-channel scales applied before quantization.
    """
    if scales is not None:
        wdw = wdw * scales[:, None, None]
    pad = ksz // 2
    wi = np.arange(128)[:, None]
    wo = np.arange(128)[None, :]
    idx = wi - wo + pad
    valid = (idx >= 0) & (idx < ksz)
    idxc = np.clip(idx, 0, ksz - 1)
    T = wdw[:, :, idxc] * valid[None, None]          # [c, ksz, 128, 128]
    T = T.reshape(-1, 128, 128).transpose(1, 0, 2)   # [128, c*ksz, 128]
    return np.ascontiguousarray(T.astype(dtype))


def _split_excess_waits(nc, limit=1):
    """This container's walrus rejects >1 sync wait per instruction (and any
    wait on Drain beyond its own barrier). Hoist extras onto same-engine
    NoOps placed immediately before."""
    import bass_rust
    import concourse.mybir as mybir

    n_split = 0
    for fn in nc.m.functions:
        for bb in fn.blocks:
            insts = bb.instructions
            i = 0
            while i < len(insts):
                inst = insts[i]
                si = inst.sync_info
                lim = 0 if type(inst).__name__ == "InstDrain" else limit
                if si is not None and si.on_wait and len(si.on_wait) > lim:
                    waits = list(si.on_wait)
                    keep, extra = waits[:lim], waits[lim:]
                    pos = i
                    for j in range(0, len(extra), max(limit, 1)):
                        ch = extra[j : j + max(limit, 1)]
                        nop = mybir.InstNoOp(
                            name=f"waitsplit_{n_split}_{pos}",
                            engine=inst.engine,
                            ins=[],
                            outs=[],
                            sync_info=bass_rust.SyncInfo(on_wait=ch, on_update=[]),
                        )
                        insts.insert(pos, nop)
                        pos += 1
                        n_split += 1
                    inst.sync_info = bass_rust.SyncInfo(
                        on_wait=keep, on_update=list(si.on_update)
                    )
                    i = pos + 1
                else:
                    i += 1
    return n_split


def _build_program():
    import concourse.bass as bass
    import concourse.mybir as mybir
    import concourse.tile as tile

    F32 = mybir.dt.float32
    BF16 = mybir.dt.bfloat16
    F8 = mybir.dt.float8e4
    F8E3 = mybir.dt.float8e3
    AF = mybir.ActivationFunctionType
    OP = mybir.AluOpType

    nc = bass.Bass("TRN2", target_bir_lowering=False, debug=False, num_devices=8)

    # ---- DRAM parameters ----
    xin = nc.dram_tensor("x", [C, HW], BF16, kind="ExternalInput").ap()
    yin = nc.dram_tensor("y", [C, HW], BF16, kind="ExternalInput").ap()
    wq_d = nc.dram_tensor("wq", [C, C], BF16, kind="ExternalInput").ap()
    wkv_d = nc.dram_tensor("wkv", [C, 384], BF16, kind="ExternalInput").ap()
    wp_d = nc.dram_tensor("wp", [2, PC, C], BF16, kind="ExternalInput").ap()
    tq_d = nc.dram_tensor("tq", [128, C * 3, 128], F8, kind="ExternalInput").ap()
    tk_d = nc.dram_tensor("tk", [128, C * 7, 128], F8, kind="ExternalInput").ap()
    tv_d = nc.dram_tensor("tv", [128, C * 7, 128], F8E3, kind="ExternalInput").ap()
    idb_d = nc.dram_tensor("idb", [128, 128], BF16, kind="ExternalInput").ap()
    mask_d = nc.dram_tensor("maskbd", [PC, PC], F32, kind="ExternalInput").ap()
    temp_d = nc.dram_tensor("temprow", [1, C], F32, kind="ExternalInput").ap()
    svinv_d = nc.dram_tensor("svinv", [2, PC, 1], F32, kind="ExternalInput").ap()
    out_d = nc.dram_tensor("out", [C, HW], F32, kind="ExternalOutput").ap()

    with tile.TileContext(nc) as tc:
        import contextlib

        with contextlib.ExitStack() as ctx:
            consts = ctx.enter_context(tc.tile_pool(name="consts", bufs=1))
            s1 = ctx.enter_context(tc.tile_pool(name="s1", bufs=1))
            s2 = ctx.enter_context(tc.tile_pool(name="s2", bufs=1))
            s3 = ctx.enter_context(tc.tile_pool(name="s3", bufs=1))
            streams = ctx.enter_context(tc.tile_pool(name="streams", bufs=2))
            tpool8 = ctx.enter_context(tc.tile_pool(name="tpool8", bufs=4))
            tpoolb = ctx.enter_context(tc.tile_pool(name="tpoolb", bufs=3))
            scratch = ctx.enter_context(tc.tile_pool(name="scratch", bufs=2))
            sqpool = ctx.enter_context(tc.tile_pool(name="sqpool", bufs=1))
            ostage = ctx.enter_context(tc.tile_pool(name="ostage", bufs=2))
            misc = ctx.enter_context(tc.tile_pool(name="misc", bufs=2))
            stats = ctx.enter_context(tc.tile_pool(name="stats", bufs=1))
            vtpool = ctx.enter_context(tc.tile_pool(name="vtpool", bufs=1))

            # ---- load constants ----
            wq0 = consts.tile([128, C], BF16)
            wq1 = consts.tile([64, C], BF16)
            nc.sync.dma_start(out=wq0, in_=wq_d[0:128, :])
            nc.sync.dma_start(out=wq1, in_=wq_d[128:192, :])
            wkv0 = consts.tile([128, 384], BF16)
            wkv1 = consts.tile([64, 384], BF16)
            nc.sync.dma_start(out=wkv0, in_=wkv_d[0:128, :])
            nc.sync.dma_start(out=wkv1, in_=wkv_d[128:192, :])
            wp0 = consts.tile([PC, C], BF16)
            wp1 = consts.tile([PC, C], BF16)
            nc.sync.dma_start(out=wp0, in_=wp_d[0])
            nc.sync.dma_start(out=wp1, in_=wp_d[1])
            identb = consts.tile([128, 128], BF16)
            nc.sync.dma_start(out=identb, in_=idb_d)
            maskbd = consts.tile([PC, PC], F32)
            nc.sync.dma_start(out=maskbd, in_=mask_d)
            temprow = consts.tile([1, C], F32)
            nc.sync.dma_start(out=temprow, in_=temp_d)
            svinv0 = consts.tile([PC, 1], F32)
            svinv1 = consts.tile([PC, 1], F32)
            nc.sync.dma_start(out=svinv0, in_=svinv_d[0])
            nc.sync.dma_start(out=svinv1, in_=svinv_d[1])
            onescol = consts.tile([128, 1], BF16)
            nc.vector.memset(onescol, 1.0)
            ones1 = consts.tile([1, 128], BF16)
            nc.vector.memset(ones1, 1.0)
            warm = consts.tile([128, 512], BF16)
            nc.vector.memset(warm, 0.5)

            # ---- big SBUF regions ----
            # channel-major: [w partitions, c*128 + h]; dw moving is contiguous
            bq = s1.tile([128, Himg * C], F8, tag="qv")
            bk = s2.tile([128, Himg * C + 32], F8, tag="kk")
            bv = s3.tile([128, Himg * C], BF16, tag="vv")
            bq3 = bq.rearrange("p (c h) -> p c h", h=Himg)
            bk3 = bk[:, 0 : Himg * C].rearrange("p (c h) -> p c h", h=Himg)
            bv3 = bv.rearrange("p (c h) -> p c h", h=Himg)
            # transposed views: [w, h, c] (strided in c)
            bq_hc = bq.rearrange("p (c h) -> p h c", h=Himg)
            bk_hc = bk[:, 0 : Himg * C].rearrange("p (c h) -> p h c", h=Himg)
            bv_hc = bv.rearrange("p (c h) -> p h c", h=Himg)

            # h-major staging rings for phase A
            HSTG, CHUNK = 32, 16
            stg = stats.tile([128, HSTG * 384], BF16, name="stg")
            stg_hc = stg.rearrange("p (h c) -> p h c", c=384)
            stg_ch = stg.rearrange("p (h c) -> p c h", c=384)
            HSTGQ = 16
            stgq = stats.tile([128, HSTGQ * C], BF16, name="stgq")
            stgq_hc = stgq.rearrange("p (h c) -> p h c", c=C)
            stgq_ch = stgq.rearrange("p (h c) -> p c h", c=C)

            partials = stats.tile([128, 2 * C], F32)
            partials_bf = stats.tile([128, 2 * C], BF16)

            # pair-0 v^T region for the first NB0 blocks
            vtb0 = vtpool.tile([PC, NB0 * 512], BF16)

            def chan_ap(region3, c, col0, cnt):
                # [128, cnt] contiguous view: channel c, h col0..col0+cnt
                return region3[:, c, col0 : col0 + cnt]

            def copy_on(eng_idx, dst, src):
                if eng_idx == 0:
                    nc.vector.tensor_copy(dst, src)
                elif eng_idx == 1:
                    nc.scalar.activation(out=dst, in_=src, func=AF.Copy)
                else:
                    nc.gpsimd.tensor_copy(dst, src)

            # ================= Phase W+A: warmup + both 1x1 convs ===========
            with tc.tile_pool(name="psA_q", bufs=3, space="PSUM") as psA_q, \
                 tc.tile_pool(name="psA_kv", bufs=5, space="PSUM") as psA_kv:
                # warmup: dense matmul burst to lift the HAM clock gate while
                # the first input slabs are still in flight
                wps = psA_kv.tile([128, 512], F32, tag="ps", name="warmps")
                for i in range(12):
                    nc.tensor.matmul(
                        wps, warm[:, 0:128], warm, start=True, stop=True
                    )

                ptq, ptkv = [None], [None]
                for h in range(Himg):
                    sl = h % SLAB
                    if sl == 0:
                        qs0 = streams.tile([128, SLAB * 128], BF16, tag="qs0")
                        qs1 = streams.tile([64, SLAB * 128], BF16, tag="qs1")
                        nc.sync.dma_start(out=qs0, in_=xin[0:128, h * 128 : (h + SLAB) * 128])
                        nc.sync.dma_start(out=qs1, in_=xin[128:192, h * 128 : (h + SLAB) * 128])
                        ys0 = streams.tile([128, SLAB * 128], BF16, tag="ys0")
                        ys1 = streams.tile([64, SLAB * 128], BF16, tag="ys1")
                        nc.sync.dma_start(out=ys0, in_=yin[0:128, h * 128 : (h + SLAB) * 128])
                        nc.sync.dma_start(out=ys1, in_=yin[128:192, h * 128 : (h + SLAB) * 128])
                    # ---- q conv (2 rows per PSUM tile) ----
                    if h % 2 == 0:
                        ptq[0] = psA_q.tile([128, 2 * C], F32, tag="tp", name=f"ptq_{h}")
                    offq = (h % 2) * C
                    nc.tensor.matmul(
                        ptq[0][:, offq : offq + C],
                        qs0[:, sl * 128 : (sl + 1) * 128], wq0,
                        start=True, stop=False,
                    )
                    nc.tensor.matmul(
                        ptq[0][:, offq : offq + C],
                        qs1[:, sl * 128 : (sl + 1) * 128], wq1,
                        start=False, stop=True,
                    )
                    # ---- kv conv (1 row per PSUM tile) ----
                    ptkv[0] = psA_kv.tile([128, 384], F32, tag="ps", name=f"ptkv_{h}")
                    nc.tensor.matmul(
                        ptkv[0], ys0[:, sl * 128 : (sl + 1) * 128], wkv0,
                        start=True, stop=False,
                    )
                    nc.tensor.matmul(
                        ptkv[0], ys1[:, sl * 128 : (sl + 1) * 128], wkv1,
                        start=False, stop=True,
                    )
                    # ---- staged evictions, halves on DVE + ACT ----
                    hs = h % HSTG
                    nc.vector.tensor_copy(stg_hc[:, hs, 0:192], ptkv[0][:, 0:192])
                    nc.scalar.activation(
                        out=stg_hc[:, hs, 192:384], in_=ptkv[0][:, 192:384],
                        func=AF.Copy,
                    )
                    if h % 2 == 1:
                        qs_ = (h - 1) % HSTGQ
                        nc.vector.tensor_copy(stgq_hc[:, qs_, :], ptq[0][:, 0:C])
                        nc.scalar.activation(
                            out=stgq_hc[:, qs_ + 1, :], in_=ptq[0][:, C : 2 * C],
                            func=AF.Copy,
                        )
                    # ---- q repack every 8 rows (DVE/ACT halves) ----
                    if h % 8 == 7:
                        hq0 = h - 7
                        sq0 = hq0 % HSTGQ
                        eq = (h // 8) % 2
                        copy_on(eq, bq3[:, 0:96, hq0 : hq0 + 8], stgq_ch[:, 0:96, sq0 : sq0 + 8])
                        copy_on(1 - eq, bq3[:, 96:192, hq0 : hq0 + 8], stgq_ch[:, 96:192, sq0 : sq0 + 8])
                    # ---- kv repack every CHUNK rows: DVE 3, ACT 3, gpsimd 2
                    if h % CHUNK == CHUNK - 1:
                        hc0 = h - CHUNK + 1
                        s0 = hc0 % HSTG
                        pattern = [0, 1, 2, 0, 1, 0, 1, 2]
                        pi = 0
                        for reg3, c0, c1 in ((bk3, 0, C), (bv3, C, 2 * C)):
                            for qi in range(4):
                                cl = c0 + qi * 48
                                copy_on(
                                    pattern[pi],
                                    reg3[:, cl - c0 : cl - c0 + 48, hc0 : hc0 + CHUNK],
                                    stg_ch[:, cl : cl + 48, s0 : s0 + CHUNK],
                                )
                                pi += 1

            # ================= Phases B..G ==================================
            GB = 4                      # channels per PSUM bank group

            with tc.tile_pool(name="ps", bufs=4, space="PSUM") as ps, \
                 tc.tile_pool(name="pst", bufs=2, space="PSUM") as pst, \
                 tc.tile_pool(name="attnp", bufs=2, space="PSUM") as attnp_pool:

                def dw_phase(region3, t_dram, ksz, tdt, pool, tag, extra=None):
                    pad = ksz // 2
                    order = [pad] + [d for d in range(ksz) if d != pad]
                    cw = 4                      # channels per T-wave
                    wave_tile = [None]
                    pdw4 = [None]
                    for ci in range(C):
                        if ci % cw == 0:
                            nt = min(cw, C - ci) * ksz
                            wave_tile[0] = pool.tile(
                                [128, cw * ksz, 128], tdt, tag=tag,
                                name=f"tw_{tag}_{ci}",
                            )
                            i0 = ci * ksz
                            nc.sync.dma_start(
                                out=wave_tile[0][:, 0:nt, :],
                                in_=t_dram[:, i0 : i0 + nt, :],
                            )
                        tw = wave_tile[0]
                        if ci % GB == 0:
                            pdw4[0] = ps.tile([128, GB * 128], F32, tag="ps", name=f"pdw_{tag}_{ci}")
                        base = (ci % cw) * ksz
                        slot = (ci % GB) * 128
                        for j, dh in enumerate(order):
                            sh = dh - pad
                            cnt = Himg - abs(sh)
                            h0o, h0i = max(0, -sh), max(0, sh)
                            nc.tensor.matmul(
                                pdw4[0][:, slot + h0o : slot + h0o + cnt],
                                tw[:, base + dh, :],
                                chan_ap(region3, ci, h0i, cnt),
                                start=(j == 0),
                                stop=(j == len(order) - 1),
                            )
                        if ci % GB == GB - 1:
                            g0 = ci - (GB - 1)
                            # group writeback halves: DVE + ACT in parallel
                            half = GB * 128 // 2
                            nc.vector.tensor_copy(
                                region3[:, g0 : g0 + GB // 2, :], pdw4[0][:, 0:half]
                            )
                            nc.scalar.activation(
                                out=region3[:, g0 + GB // 2 : ci + 1, :],
                                in_=pdw4[0][:, half : 2 * half],
                                func=AF.Copy,
                            )
                            if extra is not None:
                                extra(ci // GB)

                def emit_sq(region3, g, sq_off, sq_eng):
                    """sum-of-squares for channel group g of region3 ->
                    partials[:, sq_off + 4g : ...]."""
                    g0 = g * GB
                    sc = sqpool.tile([128, GB * 128], BF16, tag=f"sq{sq_off}", name=f"sq_{sq_off}_{g}")
                    src = region3[:, g0 : g0 + GB, :]
                    if sq_eng == "gpsimd":
                        nc.gpsimd.tensor_tensor(sc, src, src, op=OP.mult)
                    else:
                        nc.scalar.activation(out=sc, in_=src, func=AF.Square)
                    sc3 = sc.rearrange("p (c h) -> p c h", h=Himg)
                    nc.vector.tensor_reduce(
                        partials[:, sq_off + g0 : sq_off + g0 + GB],
                        sc3,
                        axis=mybir.AxisListType.X,
                        op=OP.add,
                    )

                # ---- B1: q depthwise (pure matmul stream) ----
                dw_phase(bq3, tq_d, 3, F8, tpool8, "tw8")

                # ---- B2: k depthwise; q+k squares lag behind ----
                def k_extra(g):
                    emit_sq(bk3, g, C, "scalar")     # k squares on ACT
                    emit_sq(bq3, g, 0, "gpsimd")     # q squares on gpsimd
                dw_phase(bk3, tk_d, 7, F8, tpool8, "tw8", extra=k_extra)
                nc.vector.tensor_copy(partials_bf, partials)

                # ---- QK^T accumulators (filled during E) ----
                attnps = [
                    attnp_pool.tile([PC, PC], F32, tag="at", name=f"attnp_{P}")
                    for P in range(2)
                ]
                qk_jobs = [(P, h) for P in range(2) for h in range(Himg)]
                qk_i = [0]

                ezs = [None, None]
                mps = [None, None]
                sm_state = {}
                nb_done = [0]

                def softmax_piece1(P):
                    prow = pst.tile([1, PC], F32, tag="tp")
                    nc.tensor.matmul(
                        prow, onescol, partials_bf[:, PC * P : PC * P + PC],
                        start=True, stop=True,
                    )
                    sq_row = misc.tile([1, PC], F32, tag="m1")
                    nc.scalar.activation(out=sq_row, in_=prow, func=AF.Sqrt)
                    rq_row = misc.tile([1, PC], F32, tag="m2")
                    nc.vector.reciprocal(rq_row, sq_row)
                    nc.vector.tensor_tensor(
                        rq_row, rq_row, temprow[:, PC * P : PC * P + PC], op=OP.mult
                    )
                    rq_bf = misc.tile([1, PC], BF16, tag="m3")
                    nc.vector.tensor_copy(rq_bf, rq_row)
                    pcol = pst.tile([PC, 1], F32, tag="tp")
                    nc.tensor.matmul(
                        pcol, partials_bf[:, C + PC * P : C + PC * P + PC], onescol,
                        start=True, stop=True,
                    )
                    sq_col = misc.tile([PC, 1], F32, tag="m4")
                    nc.scalar.activation(out=sq_col, in_=pcol, func=AF.Sqrt)
                    rk_col = misc.tile([PC, 1], F32, tag="m5")
                    nc.vector.reciprocal(rk_col, sq_col)
                    return rq_bf, rk_col

                def softmax_piece2(P, rq_bf, rk_col):
                    attnp = attnps[P]
                    prep = pst.tile([PC, PC], F32, tag="tp")
                    nc.tensor.matmul(
                        prep, ones1[:, 0:PC], rq_bf, start=True, stop=True
                    )
                    rqrep = misc.tile([PC, PC], F32, tag="m6")
                    nc.vector.tensor_copy(rqrep, prep)
                    t1 = misc.tile([PC, PC], F32, tag="m7")
                    nc.vector.tensor_tensor(t1, attnp, rqrep, op=OP.mult)
                    e1 = misc.tile([PC, PC], F32, tag="m8")
                    nc.scalar.activation(out=e1, in_=t1, func=AF.Exp, scale=rk_col)
                    ezero = stats.tile([PC, 128], BF16, tag=f"ez{P}")
                    nc.vector.memset(ezero[:, PC:128], 0.0)
                    nc.vector.tensor_tensor(ezero[:, 0:PC], e1, maskbd, op=OP.mult)
                    pcs = pst.tile([PC, 1], F32, tag="tp")
                    nc.tensor.matmul(
                        pcs, ezero[:, 0:PC], onescol[0:PC], start=True, stop=True
                    )
                    recip = stats.tile([PC, 1], F32, tag=f"rc{P}")
                    nc.vector.reciprocal(recip, pcs)
                    ezs[P] = (ezero, recip)

                def mps_prep(P):
                    ezero, recip = ezs[P]
                    ezt_ps = pst.tile([PC, PC], BF16, tag="tp")
                    nc.tensor.transpose(ezt_ps, ezero[:, 0:PC], identb[0:PC, 0:PC])
                    ezt = misc.tile([PC, PC], BF16, tag="m9")
                    nc.vector.tensor_copy(ezt, ezt_ps)
                    wsc = misc.tile([PC, C], BF16, tag="m10")
                    nc.vector.tensor_scalar_mul(wsc, (wp0, wp1)[P], recip)
                    pmp = ps.tile([PC, C], F32, tag="ps")
                    nc.tensor.matmul(pmp, ezt, wsc, start=True, stop=True)
                    mp = stats.tile([PC, C], BF16, tag=f"mp{P}")
                    # descale the fp8e3m4-scaled v channels (partition dim = d)
                    nc.vector.tensor_scalar_mul(mp, pmp, (svinv0, svinv1)[P])
                    mps[P] = mp

                def emit_vtb0(nb):
                    ptv = pst.tile([PC, 512], BF16, tag="tp", name=f"ptv0_{nb}")
                    for hh in range(4):
                        nc.tensor.transpose(
                            ptv[:, hh * 128 : (hh + 1) * 128],
                            bv_hc[:, nb * 4 + hh, 0:PC],
                            identb,
                        )
                    nc.vector.tensor_copy(
                        vtb0[:, nb * 512 : (nb + 1) * 512], ptv
                    )

                def v_extra(g):
                    # sprinkle QK^T matmuls through the dense v-dw stream
                    end = min(len(qk_jobs), (g + 1) * 6)
                    while qk_i[0] < end:
                        P, h = qk_jobs[qk_i[0]]
                        nc.tensor.matmul(
                            attnps[P],
                            bk_hc[:, h, PC * P : PC * P + PC],
                            bq_hc[:, h, PC * P : PC * P + PC],
                            start=(h == 0),
                            stop=(h == Himg - 1),
                        )
                        qk_i[0] += 1
                    if g == 43:
                        sm_state[0] = softmax_piece1(0)
                    elif g == 44:
                        softmax_piece2(0, *sm_state[0])
                        sm_state[1] = softmax_piece1(1)
                    elif g == 45:
                        softmax_piece2(1, *sm_state[1])
                    elif g == 46:
                        mps_prep(0)
                    elif g == 47:
                        mps_prep(1)
                    if g >= 24:
                        j = g - 24 + 1          # 1..24
                        target = j * NB0 // 24
                        while nb_done[0] < target:
                            emit_vtb0(nb_done[0])
                            nb_done[0] += 1

                # ---- E: v depthwise with everything interleaved ----
                dw_phase(bv3, tv_d, 7, F8E3, tpoolb, "twb", extra=v_extra)
                while nb_done[0] < NB0:
                    emit_vtb0(nb_done[0])
                    nb_done[0] += 1

                # ---- G tail: remaining transposes + fused (attn@v)+proj ----
                for nb in range(Himg // 4):
                    h0 = nb * 4
                    # warm-keeper: real matmul so the HAM clock stays open
                    wk = ps.tile([128, 512], F32, tag="ps", name=f"wk_{nb}")
                    nc.tensor.matmul(wk, warm[:, 0:128], warm, start=True, stop=True)
                    ptv = pst.tile([PC, 512], BF16, tag="tp", name=f"ptv1_{nb}")
                    for hh in range(4):
                        nc.tensor.transpose(
                            ptv[:, hh * 128 : (hh + 1) * 128],
                            bv_hc[:, h0 + hh, PC : PC + PC],
                            identb,
                        )
                    vtb1 = scratch.tile([PC, 512], BF16, tag="vtb1", name=f"vtb1_{nb}")
                    nc.vector.tensor_copy(vtb1, ptv)
                    if nb >= NB0:
                        ptv0 = pst.tile([PC, 512], BF16, tag="tp", name=f"ptv0b_{nb}")
                        for hh in range(4):
                            nc.tensor.transpose(
                                ptv0[:, hh * 128 : (hh + 1) * 128],
                                bv_hc[:, h0 + hh, 0:PC],
                                identb,
                            )
                        vtb0b = scratch.tile([PC, 512], BF16, tag="vtb0b", name=f"vtb0b_{nb}")
                        nc.scalar.activation(out=vtb0b, in_=ptv0, func=AF.Copy)
                        vsrc0 = vtb0b
                    else:
                        vsrc0 = vtb0[:, nb * 512 : (nb + 1) * 512]
                    n = nb * 512
                    for mi, (r0, r1) in enumerate(((0, 128), (128, 192))):
                        mw = r1 - r0
                        po = ps.tile([mw, 512], F32, tag="ps", name=f"po_{mi}_{nb}")
                        nc.tensor.matmul(
                            po, mps[0][:, r0:r1], vsrc0,
                            start=True, stop=False,
                        )
                        nc.tensor.matmul(
                            po, mps[1][:, r0:r1], vtb1,
                            start=False, stop=True,
                        )
                        so = ostage.tile([mw, 512], F32, tag="os", name=f"so_{mi}_{nb}")
                        copy_on(mi, so, po)
                        nc.sync.dma_start(out=out_d[r0:r1, n : n + 512], in_=so)

    _split_excess_waits(nc)
    return nc


def _get_program():
    global _PROG
    if _PROG is None:
        _PROG = _build_program()
    return _PROG


def kernel(x, y, q_w, q_dw_w, kv_w, kv_dw_w, proj_w, temperature):
    return _run(x, y, q_w, q_dw_w, kv_w, kv_dw_w, proj_w, temperature)[0]


def _run(x, y, q_w, q_dw_w, kv_w, kv_dw_w, proj_w, temperature, trace=False):
    from concourse.bass_utils import run_bass_kernel_spmd

    x = np.asarray(x, dtype=np.float32).reshape(B, C, HW).astype(ml_dtypes.bfloat16)
    y = np.asarray(y, dtype=np.float32).reshape(B, C, HW).astype(ml_dtypes.bfloat16)
    q_w = np.asarray(q_w, dtype=np.float32)
    kv_w = np.asarray(kv_w, dtype=np.float32)
    proj_w = np.asarray(proj_w, dtype=np.float32)
    q_dw_w = np.asarray(q_dw_w, dtype=np.float32)
    kv_dw_w = np.asarray(kv_dw_w, dtype=np.float32)
    temperature = np.asarray(temperature, dtype=np.float32).reshape(HEADS)

    wq = np.ascontiguousarray(q_w[:, :, 0, 0].T.astype(ml_dtypes.bfloat16))
    wkv = np.ascontiguousarray(kv_w[:, :, 0, 0].T.astype(ml_dtypes.bfloat16))  # [C, 2C]
    # v depthwise in fp8e3m4 with per-channel power-of-2 scaling; inverse
    # scales ride a [PC,1] per-partition multiply on the fused projection
    wv = kv_dw_w[C : 2 * C, 0]                              # [C, 7, 7]
    v_absmax = np.abs(wv).reshape(C, -1).max(axis=1)
    v_scale = 2.0 ** np.floor(np.log2(14.0 / v_absmax))
    svinv = (1.0 / v_scale).astype(np.float32).reshape(2, PC, 1)
    wpT = proj_w[:, :, 0, 0].T                              # [c_in, c_out]
    wp = np.stack([wpT[0:PC], wpT[PC:C]]).astype(ml_dtypes.bfloat16)
    tq = _build_toeplitz(q_dw_w[:, 0], 3, ml_dtypes.float8_e4m3)
    tk = _build_toeplitz(kv_dw_w[0:C, 0], 7, ml_dtypes.float8_e4m3)
    tv = _build_toeplitz(wv, 7, ml_dtypes.float8_e3m4, scales=v_scale)
    idb = np.eye(128, dtype=ml_dtypes.bfloat16)
    maskbd = np.zeros((PC, PC), np.float32)
    maskbd[0:DHC, 0:DHC] = 1.0
    maskbd[DHC:PC, DHC:PC] = 1.0
    temprow = np.repeat(temperature, DHC).reshape(1, C)

    shared = {
        "wq": wq, "wkv": wkv, "wp": wp, "tq": tq, "tk": tk, "tv": tv,
        "idb": idb, "maskbd": maskbd, "temprow": temprow, "svinv": svinv,
    }
    in_maps = [dict(shared, x=x[i], y=y[i]) for i in range(B)]

    nc = _get_program()
    res = run_bass_kernel_spmd(
        nc, in_maps, core_ids=list(range(B)), trace=trace
    )
    out = np.stack([res.results[i]["out"] for i in range(B)])
    return out.reshape(B, C, Himg, Wimg).astype(np.float32), res


# revision 20
# speedup vs baseline: 1.0353x; 1.0353x over previous
"""nn_CrossAttention Trainium2 kernel — 8-core data-parallel over batch.

Per core (batch slice b=1):
  Warmup matmul burst un-throttles the PE clock (HAM) while the first
  input slabs stream in.  Both 1x1 convs run interleaved in bf16
  (stationary = 8-row input slabs); h-major PSUM tiles are staged
  contiguously (eviction halves split across DVE/ACT, 3+5 PSUM banks
  deep) then repacked to channel-major SBUF regions by DVE/ACT/gpsimd.
  Depthwise 3x3/7x7 as per-(channel, dh) banded-Toeplitz matmuls on the
  TensorEngine (q/k tables fp8e4m3, v tables fp8e3m4 with per-channel
  power-of-2 scaling descaled via a per-partition multiply on the fused
  projection matrix).  Sum-of-squares for the l2 norms lag behind the
  k-dw loop (q squares on gpsimd, k squares on ACT, reduces on DVE).
  QK^T matmuls for both head-pairs are sprinkled through the v-dw
  matmul stream (their strided moving operands are slow, so they ride
  the dense stream instead of running alone and cold); softmax chains
  and half the pair-0 v^T transposes also interleave with v-dw.  The
  output tail fuses the remaining transposes with the (attn@v)+proj
  matmuls, with one warm-keeper matmul per block so the clock gate
  stays open.
"""

import sys

sys.path.insert(0, "/opt/trn_rl_repo")

import numpy as np
import ml_dtypes

B, C, Himg, Wimg = 8, 192, 128, 128
HW = Himg * Wimg
HEADS, DHC = 4, 48      # heads, channels per head
PC = 96                 # channels per head-pair
SLAB = 8                # image rows per input stream slab
NB0 = 16                # pair-0 v^T blocks transposed during v-dw

_PROG = None            # cached (nc, meta)


def _build_toeplitz(wdw, ksz, dtype, scales=None):
    """wdw [c, ksz, ksz] f32 -> [128, c*ksz, 128], tile index = c*ksz + dh.

    T[w_in, tile, w_out] = wdw[c, dh, w_in - w_out + pad] inside the band,
    else 0.  Partition-major so a wave DMA reads contiguous bytes per
    partition.  Optional per-channel scales applied before quantization.
    """
    if scales is not None:
        wdw = wdw * scales[:, None, None]
    pad = ksz // 2
    wi = np.arange(128)[:, None]
    wo = np.arange(128)[None, :]
    idx = wi - wo + pad
    valid = (idx >= 0) & (idx < ksz)
    idxc = np.clip(idx, 0, ksz - 1)
    T = wdw[:, :, idxc] * valid[None, None]          # [c, ksz, 128, 128]
    T = T.reshape(-1, 128, 128).transpose(1, 0, 2)   # [128, c*ksz, 128]
    return np.ascontiguousarray(T.astype(dtype))


def _split_excess_waits(nc, limit=1):
    """This container's walrus rejects >1 sync wait per instruction (and any
    wait on Drain beyond its own barrier). Hoist extras onto same-engine
    NoOps placed immediately before."""
    import bass_rust
    import concourse.mybir as mybir

    n_split = 0
    for fn in nc.m.functions:
        for bb in fn.blocks:
            insts = bb.instructions
            i = 0
            while i < len(insts):
                inst = insts[i]
                si = inst.sync_info
                lim = 0 if type(inst).__name__ == "InstDrain" else limit
                if si is not None and si.on_wait and len(si.on_wait) > lim:
                    waits = list(si.on_wait)
                    keep, extra = waits[:lim], waits[lim:]
                    pos = i
                    for j in range(0, len(extra), max(limit, 1)):
                        ch = extra[j : j + max(limit, 1)]
                        nop = mybir.InstNoOp(
                            name=f"waitsplit_{n_split}_{pos}",
                            engine=inst.engine,
                            ins=[],
                            outs=[],
                            sync_info=bass_rust.SyncInfo(on_wait=ch, on_update=[]),
                        )
                        insts.insert(pos, nop)
                        pos += 1
                        n_split += 1
                    inst.sync_info = bass_rust.SyncInfo(
                        on_wait=keep, on_update=list(si.on_update)
                    )
                    i = pos + 1
                else:
                    i += 1
    return n_split


def _build_program():
    import concourse.bass as bass
    import concourse.mybir as mybir
    import concourse.tile as tile

    F32 = mybir.dt.float32
    BF16 = mybir.dt.bfloat16
    F8 = mybir.dt.float8e4
    F8E3 = mybir.dt.float8e3
    AF = mybir.ActivationFunctionType
    OP = mybir.AluOpType

    nc = bass.Bass("TRN2", target_bir_lowering=False, debug=False, num_devices=8)

    # ---- DRAM parameters ----
    xin = nc.dram_tensor("x", [C, HW], BF16, kind="ExternalInput").ap()
    yin = nc.dram_tensor("y", [C, HW], BF16, kind="ExternalInput").ap()
    wq_d = nc.dram_tensor("wq", [256, C], BF16, kind="ExternalInput").ap()
    wkv_d = nc.dram_tensor("wkv", [256, 384], BF16, kind="ExternalInput").ap()
    wp_d = nc.dram_tensor("wp", [2, PC, C], BF16, kind="ExternalInput").ap()
    tq_d = nc.dram_tensor("tq", [128, C * 3, 128], F8, kind="ExternalInput").ap()
    tk_d = nc.dram_tensor("tk", [128, C * 7, 128], F8, kind="ExternalInput").ap()
    tv_d = nc.dram_tensor("tv", [128, C * 7, 128], F8E3, kind="ExternalInput").ap()
    idb_d = nc.dram_tensor("idb", [128, 128], BF16, kind="ExternalInput").ap()
    mask_d = nc.dram_tensor("maskbd", [PC, PC], F32, kind="ExternalInput").ap()
    temp_d = nc.dram_tensor("temprow", [1, C], F32, kind="ExternalInput").ap()
    svinv_d = nc.dram_tensor("svinv", [2, PC, 1], F32, kind="ExternalInput").ap()
    out_d = nc.dram_tensor("out", [C, HW], F32, kind="ExternalOutput").ap()

    with tile.TileContext(nc) as tc:
        import contextlib

        with contextlib.ExitStack() as ctx:
            consts = ctx.enter_context(tc.tile_pool(name="consts", bufs=1))
            s1 = ctx.enter_context(tc.tile_pool(name="s1", bufs=1))
            s2 = ctx.enter_context(tc.tile_pool(name="s2", bufs=1))
            s3 = ctx.enter_context(tc.tile_pool(name="s3", bufs=1))
            streams = ctx.enter_context(tc.tile_pool(name="streams", bufs=2))
            tpool8 = ctx.enter_context(tc.tile_pool(name="tpool8", bufs=4))
            tpoolb = ctx.enter_context(tc.tile_pool(name="tpoolb", bufs=3))
            scratch = ctx.enter_context(tc.tile_pool(name="scratch", bufs=2))
            sqpool = ctx.enter_context(tc.tile_pool(name="sqpool", bufs=1))
            ostage = ctx.enter_context(tc.tile_pool(name="ostage", bufs=2))
            misc = ctx.enter_context(tc.tile_pool(name="misc", bufs=2))
            stats = ctx.enter_context(tc.tile_pool(name="stats", bufs=1))
            vtpool = ctx.enter_context(tc.tile_pool(name="vtpool", bufs=1))

            # ---- load constants ----
            # second K-chunk padded to 128 rows (zeros) so every conv matmul
            # is K=128 — partial-row LDWEIGHTS serialize the PE pipeline
            wq0 = consts.tile([128, C], BF16)
            wq1 = consts.tile([128, C], BF16)
            nc.sync.dma_start(out=wq0, in_=wq_d[0:128, :])
            nc.sync.dma_start(out=wq1, in_=wq_d[128:256, :])
            wkv0 = consts.tile([128, 384], BF16)
            wkv1 = consts.tile([128, 384], BF16)
            nc.sync.dma_start(out=wkv0, in_=wkv_d[0:128, :])
            nc.sync.dma_start(out=wkv1, in_=wkv_d[128:256, :])
            wp0 = consts.tile([PC, C], BF16)
            wp1 = consts.tile([PC, C], BF16)
            nc.sync.dma_start(out=wp0, in_=wp_d[0])
            nc.sync.dma_start(out=wp1, in_=wp_d[1])
            identb = consts.tile([128, 128], BF16)
            nc.sync.dma_start(out=identb, in_=idb_d)
            maskbd = consts.tile([PC, PC], F32)
            nc.sync.dma_start(out=maskbd, in_=mask_d)
            temprow = consts.tile([1, C], F32)
            nc.sync.dma_start(out=temprow, in_=temp_d)
            svinv0 = consts.tile([PC, 1], F32)
            svinv1 = consts.tile([PC, 1], F32)
            nc.sync.dma_start(out=svinv0, in_=svinv_d[0])
            nc.sync.dma_start(out=svinv1, in_=svinv_d[1])
            onescol = consts.tile([128, 1], BF16)
            nc.vector.memset(onescol, 1.0)
            ones1 = consts.tile([1, 128], BF16)
            nc.vector.memset(ones1, 1.0)
            warm = consts.tile([128, 512], BF16)
            nc.vector.memset(warm, 0.5)

            # ---- big SBUF regions ----
            # channel-major: [w partitions, c*128 + h]; dw moving is contiguous
            bq = s1.tile([128, Himg * C], F8, tag="qv")
            bk = s2.tile([128, Himg * C + 32], F8, tag="kk")
            bv = s3.tile([128, Himg * C], BF16, tag="vv")
            bq3 = bq.rearrange("p (c h) -> p c h", h=Himg)
            bk3 = bk[:, 0 : Himg * C].rearrange("p (c h) -> p c h", h=Himg)
            bv3 = bv.rearrange("p (c h) -> p c h", h=Himg)
            # transposed views: [w, h, c] (strided in c)
            bq_hc = bq.rearrange("p (c h) -> p h c", h=Himg)
            bk_hc = bk[:, 0 : Himg * C].rearrange("p (c h) -> p h c", h=Himg)
            bv_hc = bv.rearrange("p (c h) -> p h c", h=Himg)

            # h-major staging rings for phase A
            HSTG, CHUNK = 32, 16
            stg = stats.tile([128, HSTG * 384], BF16, name="stg")
            stg_hc = stg.rearrange("p (h c) -> p h c", c=384)
            stg_ch = stg.rearrange("p (h c) -> p c h", c=384)
            HSTGQ = 16
            stgq = stats.tile([128, HSTGQ * C], BF16, name="stgq")
            stgq_hc = stgq.rearrange("p (h c) -> p h c", c=C)
            stgq_ch = stgq.rearrange("p (h c) -> p c h", c=C)

            partials = stats.tile([128, 2 * C], F32)
            partials_bf = stats.tile([128, 2 * C], BF16)

            # pair-0 v^T region for the first NB0 blocks; 128 partitions with
            # zero rows 96:128 so the fused output matmuls stay K=128
            vtb0 = vtpool.tile([128, NB0 * 512], BF16)
            nc.vector.memset(vtb0[PC:128, :], 0.0)

            def chan_ap(region3, c, col0, cnt):
                # [128, cnt] contiguous view: channel c, h col0..col0+cnt
                return region3[:, c, col0 : col0 + cnt]

            def copy_on(eng_idx, dst, src):
                if eng_idx == 0:
                    nc.vector.tensor_copy(dst, src)
                elif eng_idx == 1:
                    nc.scalar.activation(out=dst, in_=src, func=AF.Copy)
                else:
                    nc.gpsimd.tensor_copy(dst, src)

            # ================= Phase W+A: warmup + both 1x1 convs ===========
            with tc.tile_pool(name="psA_q", bufs=3, space="PSUM") as psA_q, \
                 tc.tile_pool(name="psA_kv", bufs=5, space="PSUM") as psA_kv:
                # warmup: dense matmul burst to lift the HAM clock gate while
                # the first input slabs are still in flight
                wps = psA_kv.tile([128, 512], F32, tag="ps", name="warmps")
                for i in range(12):
                    nc.tensor.matmul(
                        wps, warm[:, 0:128], warm, start=True, stop=True
                    )

                ptq, ptkv = [None], [None]
                for h in range(Himg):
                    sl = h % SLAB
                    if sl == 0:
                        qs0 = streams.tile([128, SLAB * 128], BF16, tag="qs0")
                        qs1 = streams.tile([128, SLAB * 128], BF16, tag="qs1")
                        nc.sync.dma_start(out=qs0, in_=xin[0:128, h * 128 : (h + SLAB) * 128])
                        nc.sync.dma_start(out=qs1[0:64], in_=xin[128:192, h * 128 : (h + SLAB) * 128])
                        nc.vector.memset(qs1[64:128], 0.0)
                        ys0 = streams.tile([128, SLAB * 128], BF16, tag="ys0")
                        ys1 = streams.tile([128, SLAB * 128], BF16, tag="ys1")
                        nc.sync.dma_start(out=ys0, in_=yin[0:128, h * 128 : (h + SLAB) * 128])
                        nc.sync.dma_start(out=ys1[0:64], in_=yin[128:192, h * 128 : (h + SLAB) * 128])
                        nc.vector.memset(ys1[64:128], 0.0)
                    # ---- q conv (2 rows per PSUM tile) ----
                    if h % 2 == 0:
                        ptq[0] = psA_q.tile([128, 2 * C], F32, tag="tp", name=f"ptq_{h}")
                    offq = (h % 2) * C
                    nc.tensor.matmul(
                        ptq[0][:, offq : offq + C],
                        qs0[:, sl * 128 : (sl + 1) * 128], wq0,
                        start=True, stop=False,
                    )
                    nc.tensor.matmul(
                        ptq[0][:, offq : offq + C],
                        qs1[:, sl * 128 : (sl + 1) * 128], wq1,
                        start=False, stop=True,
                    )
                    # ---- kv conv (1 row per PSUM tile) ----
                    ptkv[0] = psA_kv.tile([128, 384], F32, tag="ps", name=f"ptkv_{h}")
                    nc.tensor.matmul(
                        ptkv[0], ys0[:, sl * 128 : (sl + 1) * 128], wkv0,
                        start=True, stop=False,
                    )
                    nc.tensor.matmul(
                        ptkv[0], ys1[:, sl * 128 : (sl + 1) * 128], wkv1,
                        start=False, stop=True,
                    )
                    # ---- staged evictions, halves on DVE + ACT ----
                    hs = h % HSTG
                    nc.vector.tensor_copy(stg_hc[:, hs, 0:192], ptkv[0][:, 0:192])
                    nc.scalar.activation(
                        out=stg_hc[:, hs, 192:384], in_=ptkv[0][:, 192:384],
                        func=AF.Copy,
                    )
                    if h % 2 == 1:
                        qs_ = (h - 1) % HSTGQ
                        nc.vector.tensor_copy(stgq_hc[:, qs_, :], ptq[0][:, 0:C])
                        nc.scalar.activation(
                            out=stgq_hc[:, qs_ + 1, :], in_=ptq[0][:, C : 2 * C],
                            func=AF.Copy,
                        )
                    # ---- q repack every 8 rows (DVE/ACT halves) ----
                    if h % 8 == 7:
                        hq0 = h - 7
                        sq0 = hq0 % HSTGQ
                        eq = (h // 8) % 2
                        copy_on(eq, bq3[:, 0:96, hq0 : hq0 + 8], stgq_ch[:, 0:96, sq0 : sq0 + 8])
                        copy_on(1 - eq, bq3[:, 96:192, hq0 : hq0 + 8], stgq_ch[:, 96:192, sq0 : sq0 + 8])
                    # ---- kv repack every CHUNK rows: DVE 3, ACT 3, gpsimd 2
                    if h % CHUNK == CHUNK - 1:
                        hc0 = h - CHUNK + 1
                        s0 = hc0 % HSTG
                        pattern = [0, 1, 2, 0, 1, 0, 1, 2]
                        pi = 0
                        for reg3, c0, c1 in ((bk3, 0, C), (bv3, C, 2 * C)):
                            for qi in range(4):
                                cl = c0 + qi * 48
                                copy_on(
                                    pattern[pi],
                                    reg3[:, cl - c0 : cl - c0 + 48, hc0 : hc0 + CHUNK],
                                    stg_ch[:, cl : cl + 48, s0 : s0 + CHUNK],
                                )
                                pi += 1

            # ================= Phases B..G ==================================
            GB = 4                      # channels per PSUM bank group

            with tc.tile_pool(name="ps", bufs=4, space="PSUM") as ps, \
                 tc.tile_pool(name="pst", bufs=2, space="PSUM") as pst, \
                 tc.tile_pool(name="attnp", bufs=2, space="PSUM") as attnp_pool:

                def dw_phase(region3, t_dram, ksz, tdt, pool, tag, extra=None):
                    pad = ksz // 2
                    order = [pad] + [d for d in range(ksz) if d != pad]
                    cw = 4                      # channels per T-wave
                    wave_tile = [None]
                    pdw4 = [None]
                    for ci in range(C):
                        if ci % cw == 0:
                            nt = min(cw, C - ci) * ksz
                            wave_tile[0] = pool.tile(
                                [128, cw * ksz, 128], tdt, tag=tag,
                                name=f"tw_{tag}_{ci}",
                            )
                            i0 = ci * ksz
                            nc.sync.dma_start(
                                out=wave_tile[0][:, 0:nt, :],
                                in_=t_dram[:, i0 : i0 + nt, :],
                            )
                        tw = wave_tile[0]
                        if ci % GB == 0:
                            pdw4[0] = ps.tile([128, GB * 128], F32, tag="ps", name=f"pdw_{tag}_{ci}")
                        base = (ci % cw) * ksz
                        slot = (ci % GB) * 128
                        for j, dh in enumerate(order):
                            sh = dh - pad
                            cnt = Himg - abs(sh)
                            h0o, h0i = max(0, -sh), max(0, sh)
                            nc.tensor.matmul(
                                pdw4[0][:, slot + h0o : slot + h0o + cnt],
                                tw[:, base + dh, :],
                                chan_ap(region3, ci, h0i, cnt),
                                start=(j == 0),
                                stop=(j == len(order) - 1),
                            )
                        if ci % GB == GB - 1:
                            g0 = ci - (GB - 1)
                            # group writeback halves: DVE + ACT in parallel
                            half = GB * 128 // 2
                            nc.vector.tensor_copy(
                                region3[:, g0 : g0 + GB // 2, :], pdw4[0][:, 0:half]
                            )
                            nc.scalar.activation(
                                out=region3[:, g0 + GB // 2 : ci + 1, :],
                                in_=pdw4[0][:, half : 2 * half],
                                func=AF.Copy,
                            )
                            if extra is not None:
                                extra(ci // GB)

                def emit_sq(region3, g, sq_off, sq_eng):
                    """sum-of-squares for channel group g of region3 ->
                    partials[:, sq_off + 4g : ...]."""
                    g0 = g * GB
                    sc = sqpool.tile([128, GB * 128], BF16, tag=f"sq{sq_off}", name=f"sq_{sq_off}_{g}")
                    src = region3[:, g0 : g0 + GB, :]
                    if sq_eng == "gpsimd":
                        nc.gpsimd.tensor_tensor(sc, src, src, op=OP.mult)
                    else:
                        nc.scalar.activation(out=sc, in_=src, func=AF.Square)
                    sc3 = sc.rearrange("p (c h) -> p c h", h=Himg)
                    nc.vector.tensor_reduce(
                        partials[:, sq_off + g0 : sq_off + g0 + GB],
                        sc3,
                        axis=mybir.AxisListType.X,
                        op=OP.add,
                    )

                # ---- B1: q depthwise (pure matmul stream) ----
                dw_phase(bq3, tq_d, 3, F8, tpool8, "tw8")

                # ---- B2: k depthwise; q+k squares lag behind ----
                def k_extra(g):
                    emit_sq(bk3, g, C, "scalar")     # k squares on ACT
                    emit_sq(bq3, g, 0, "gpsimd")     # q squares on gpsimd
                dw_phase(bk3, tk_d, 7, F8, tpool8, "tw8", extra=k_extra)
                nc.vector.tensor_copy(partials_bf, partials)

                # ---- QK^T accumulators (filled during E) ----
                attnps = [
                    attnp_pool.tile([PC, PC], F32, tag="at", name=f"attnp_{P}")
                    for P in range(2)
                ]
                qk_jobs = [(P, h) for P in range(2) for h in range(Himg)]
                qk_i = [0]

                ezs = [None, None]
                mps = [None, None]
                sm_state = {}
                nb_done = [0]

                def softmax_piece1(P):
                    prow = pst.tile([1, PC], F32, tag="tp")
                    nc.tensor.matmul(
                        prow, onescol, partials_bf[:, PC * P : PC * P + PC],
                        start=True, stop=True,
                    )
                    sq_row = misc.tile([1, PC], F32, tag="m1")
                    nc.scalar.activation(out=sq_row, in_=prow, func=AF.Sqrt)
                    rq_row = misc.tile([1, PC], F32, tag="m2")
                    nc.vector.reciprocal(rq_row, sq_row)
                    nc.vector.tensor_tensor(
                        rq_row, rq_row, temprow[:, PC * P : PC * P + PC], op=OP.mult
                    )
                    rq_bf = misc.tile([1, PC], BF16, tag="m3")
                    nc.vector.tensor_copy(rq_bf, rq_row)
                    pcol = pst.tile([PC, 1], F32, tag="tp")
                    nc.tensor.matmul(
                        pcol, partials_bf[:, C + PC * P : C + PC * P + PC], onescol,
                        start=True, stop=True,
                    )
                    sq_col = misc.tile([PC, 1], F32, tag="m4")
                    nc.scalar.activation(out=sq_col, in_=pcol, func=AF.Sqrt)
                    rk_col = misc.tile([PC, 1], F32, tag="m5")
                    nc.vector.reciprocal(rk_col, sq_col)
                    return rq_bf, rk_col

                def softmax_piece2(P, rq_bf, rk_col):
                    attnp = attnps[P]
                    prep = pst.tile([PC, PC], F32, tag="tp")
                    nc.tensor.matmul(
                        prep, ones1[:, 0:PC], rq_bf, start=True, stop=True
                    )
                    rqrep = misc.tile([PC, PC], F32, tag="m6")
                    nc.vector.tensor_copy(rqrep, prep)
                    t1 = misc.tile([PC, PC], F32, tag="m7")
                    nc.vector.tensor_tensor(t1, attnp, rqrep, op=OP.mult)
                    e1 = misc.tile([PC, PC], F32, tag="m8")
                    nc.scalar.activation(out=e1, in_=t1, func=AF.Exp, scale=rk_col)
                    ezero = stats.tile([PC, 128], BF16, tag=f"ez{P}")
                    nc.vector.memset(ezero[:, PC:128], 0.0)
                    nc.vector.tensor_tensor(ezero[:, 0:PC], e1, maskbd, op=OP.mult)
                    pcs = pst.tile([PC, 1], F32, tag="tp")
                    nc.tensor.matmul(
                        pcs, ezero[:, 0:PC], onescol[0:PC], start=True, stop=True
                    )
                    recip = stats.tile([PC, 1], F32, tag=f"rc{P}")
                    nc.vector.reciprocal(recip, pcs)
                    ezs[P] = (ezero, recip)

                def mps_prep(P):
                    ezero, recip = ezs[P]
                    ezt_ps = pst.tile([PC, PC], BF16, tag="tp")
                    nc.tensor.transpose(ezt_ps, ezero[:, 0:PC], identb[0:PC, 0:PC])
                    ezt = misc.tile([PC, PC], BF16, tag="m9")
                    nc.vector.tensor_copy(ezt, ezt_ps)
                    wsc = misc.tile([PC, C], BF16, tag="m10")
                    nc.vector.tensor_scalar_mul(wsc, (wp0, wp1)[P], recip)
                    pmp = ps.tile([PC, C], F32, tag="ps")
                    nc.tensor.matmul(pmp, ezt, wsc, start=True, stop=True)
                    mp = stats.tile([128, C], BF16, tag=f"mp{P}")
                    nc.vector.memset(mp[PC:128, :], 0.0)
                    # descale the fp8e3m4-scaled v channels (partition dim = d)
                    nc.vector.tensor_scalar_mul(mp[0:PC, :], pmp, (svinv0, svinv1)[P])
                    mps[P] = mp

                def emit_vtb0(nb):
                    ptv = pst.tile([PC, 512], BF16, tag="tp", name=f"ptv0_{nb}")
                    for hh in range(4):
                        nc.tensor.transpose(
                            ptv[:, hh * 128 : (hh + 1) * 128],
                            bv_hc[:, nb * 4 + hh, 0:PC],
                            identb,
                        )
                    nc.vector.tensor_copy(
                        vtb0[0:PC, nb * 512 : (nb + 1) * 512], ptv
                    )

                def v_extra(g):
                    # sprinkle QK^T matmuls through the dense v-dw stream
                    end = min(len(qk_jobs), (g + 1) * 6)
                    while qk_i[0] < end:
                        P, h = qk_jobs[qk_i[0]]
                        nc.tensor.matmul(
                            attnps[P],
                            bk_hc[:, h, PC * P : PC * P + PC],
                            bq_hc[:, h, PC * P : PC * P + PC],
                            start=(h == 0),
                            stop=(h == Himg - 1),
                        )
                        qk_i[0] += 1
                    if g == 43:
                        sm_state[0] = softmax_piece1(0)
                    elif g == 44:
                        softmax_piece2(0, *sm_state[0])
                        sm_state[1] = softmax_piece1(1)
                    elif g == 45:
                        softmax_piece2(1, *sm_state[1])
                    elif g == 46:
                        mps_prep(0)
                    elif g == 47:
                        mps_prep(1)
                    if g >= 24:
                        j = g - 24 + 1          # 1..24
                        target = j * NB0 // 24
                        while nb_done[0] < target:
                            emit_vtb0(nb_done[0])
                            nb_done[0] += 1

                # ---- E: v depthwise with everything interleaved ----
                dw_phase(bv3, tv_d, 7, F8E3, tpoolb, "twb", extra=v_extra)
                while nb_done[0] < NB0:
                    emit_vtb0(nb_done[0])
                    nb_done[0] += 1

                # ---- G tail: remaining transposes + fused (attn@v)+proj ----
                for nb in range(Himg // 4):
                    h0 = nb * 4
                    # warm-keeper: real matmul so the HAM clock stays open
                    wk = ps.tile([128, 512], F32, tag="ps", name=f"wk_{nb}")
                    nc.tensor.matmul(wk, warm[:, 0:128], warm, start=True, stop=True)
                    ptv = pst.tile([PC, 512], BF16, tag="tp", name=f"ptv1_{nb}")
                    for hh in range(4):
                        nc.tensor.transpose(
                            ptv[:, hh * 128 : (hh + 1) * 128],
                            bv_hc[:, h0 + hh, PC : PC + PC],
                            identb,
                        )
                    vtb1 = scratch.tile([128, 512], BF16, tag="vtb1", name=f"vtb1_{nb}")
                    nc.vector.memset(vtb1[PC:128, :], 0.0)
                    nc.vector.tensor_copy(vtb1[0:PC, :], ptv)
                    if nb >= NB0:
                        ptv0 = pst.tile([PC, 512], BF16, tag="tp", name=f"ptv0b_{nb}")
                        for hh in range(4):
                            nc.tensor.transpose(
                                ptv0[:, hh * 128 : (hh + 1) * 128],
                                bv_hc[:, h0 + hh, 0:PC],
                                identb,
                            )
                        vtb0b = scratch.tile([128, 512], BF16, tag="vtb0b", name=f"vtb0b_{nb}")
                        nc.vector.memset(vtb0b[PC:128, :], 0.0)
                        nc.scalar.activation(out=vtb0b[0:PC, :], in_=ptv0, func=AF.Copy)
                        vsrc0 = vtb0b
                    else:
                        vsrc0 = vtb0[:, nb * 512 : (nb + 1) * 512]
                    n = nb * 512
                    for mi, (r0, r1) in enumerate(((0, 128), (128, 192))):
                        mw = r1 - r0
                        po = ps.tile([mw, 512], F32, tag="ps", name=f"po_{mi}_{nb}")
                        nc.tensor.matmul(
                            po, mps[0][:, r0:r1], vsrc0,
                            start=True, stop=False,
                        )
                        nc.tensor.matmul(
                            po, mps[1][:, r0:r1], vtb1,
                            start=False, stop=True,
                        )
                        so = ostage.tile([mw, 512], F32, tag="os", name=f"so_{mi}_{nb}")
                        copy_on(mi, so, po)
                        nc.sync.dma_start(out=out_d[r0:r1, n : n + 512], in_=so)

    _split_excess_waits(nc)
    return nc


def _get_program():
    global _PROG
    if _PROG is None:
        _PROG = _build_program()
    return _PROG


def kernel(x, y, q_w, q_dw_w, kv_w, kv_dw_w, proj_w, temperature):
    return _run(x, y, q_w, q_dw_w, kv_w, kv_dw_w, proj_w, temperature)[0]


def _run(x, y, q_w, q_dw_w, kv_w, kv_dw_w, proj_w, temperature, trace=False):
    from concourse.bass_utils import run_bass_kernel_spmd

    x = np.asarray(x, dtype=np.float32).reshape(B, C, HW).astype(ml_dtypes.bfloat16)
    y = np.asarray(y, dtype=np.float32).reshape(B, C, HW).astype(ml_dtypes.bfloat16)
    q_w = np.asarray(q_w, dtype=np.float32)
    kv_w = np.asarray(kv_w, dtype=np.float32)
    proj_w = np.asarray(proj_w, dtype=np.float32)
    q_dw_w = np.asarray(q_dw_w, dtype=np.float32)
    kv_dw_w = np.asarray(kv_dw_w, dtype=np.float32)
    temperature = np.asarray(temperature, dtype=np.float32).reshape(HEADS)

    # conv weights padded to 256 K-rows (zeros) for K=128-homogeneous matmuls
    wq = np.zeros((256, C), np.float32)
    wq[0:C] = q_w[:, :, 0, 0].T
    wq = wq.astype(ml_dtypes.bfloat16)
    wkv = np.zeros((256, 384), np.float32)
    wkv[0:C] = kv_w[:, :, 0, 0].T
    wkv = wkv.astype(ml_dtypes.bfloat16)
    # v depthwise in fp8e3m4 with per-channel power-of-2 scaling; inverse
    # scales ride a [PC,1] per-partition multiply on the fused projection
    wv = kv_dw_w[C : 2 * C, 0]                              # [C, 7, 7]
    v_absmax = np.abs(wv).reshape(C, -1).max(axis=1)
    v_scale = 2.0 ** np.floor(np.log2(14.0 / v_absmax))
    svinv = (1.0 / v_scale).astype(np.float32).reshape(2, PC, 1)
    wpT = proj_w[:, :, 0, 0].T                              # [c_in, c_out]
    wp = np.stack([wpT[0:PC], wpT[PC:C]]).astype(ml_dtypes.bfloat16)
    tq = _build_toeplitz(q_dw_w[:, 0], 3, ml_dtypes.float8_e4m3)
    tk = _build_toeplitz(kv_dw_w[0:C, 0], 7, ml_dtypes.float8_e4m3)
    tv = _build_toeplitz(wv, 7, ml_dtypes.float8_e3m4, scales=v_scale)
    idb = np.eye(128, dtype=ml_dtypes.bfloat16)
    maskbd = np.zeros((PC, PC), np.float32)
    maskbd[0:DHC, 0:DHC] = 1.0
    maskbd[DHC:PC, DHC:PC] = 1.0
    temprow = np.repeat(temperature, DHC).reshape(1, C)

    shared = {
        "wq": wq, "wkv": wkv, "wp": wp, "tq": tq, "tk": tk, "tv": tv,
        "idb": idb, "maskbd": maskbd, "temprow": temprow, "svinv": svinv,
    }
    in_maps = [dict(shared, x=x[i], y=y[i]) for i in range(B)]

    nc = _get_program()
    res = run_bass_kernel_spmd(
        nc, in_maps, core_ids=list(range(B)), trace=trace
    )
    out = np.stack([res.results[i]["out"] for i in range(B)])
    return out.reshape(B, C, Himg, Wimg).astype(np.float32), res


# revision 29
# speedup vs baseline: 1.0833x; 1.0463x over previous
"""nn_CrossAttention Trainium2 kernel — 8-core data-parallel over batch.

Per core (batch slice b=1):
  Warmup matmul burst un-throttles the PE clock (HAM) while the first
  input slabs stream in.  Both 1x1 convs run interleaved in bf16
  (stationary = 8-row input slabs); h-major PSUM tiles are staged
  contiguously (eviction halves split across DVE/ACT, 3+5 PSUM banks
  deep) then repacked to channel-major SBUF regions by DVE/ACT/gpsimd.
  Depthwise 3x3/7x7 as per-(channel, dh) banded-Toeplitz matmuls on the
  TensorEngine (q/k tables fp8e4m3, v tables fp8e3m4 with per-channel
  power-of-2 scaling descaled via a per-partition multiply on the fused
  projection matrix).  Sum-of-squares for the l2 norms lag behind the
  k-dw loop (q squares on gpsimd, k squares on ACT, reduces on DVE).
  QK^T matmuls for both head-pairs are sprinkled through the v-dw
  matmul stream (their strided moving operands are slow, so they ride
  the dense stream instead of running alone and cold); softmax chains
  and half the pair-0 v^T transposes also interleave with v-dw.  The
  output tail fuses the remaining transposes with the (attn@v)+proj
  matmuls, with one warm-keeper matmul per block so the clock gate
  stays open.
"""

import sys

sys.path.insert(0, "/opt/trn_rl_repo")

import numpy as np
import ml_dtypes

B, C, Himg, Wimg = 8, 192, 128, 128
HW = Himg * Wimg
HEADS, DHC = 4, 48      # heads, channels per head
PC = 96                 # channels per head-pair
SLAB = 8                # image rows per input stream slab
NB0 = 16                # pair-0 v^T blocks transposed during v-dw

_PROG = None            # cached (nc, meta)


def _build_toeplitz(wdw, ksz, dtype, scales=None):
    """wdw [c, ksz, ksz] f32 -> [128, c*ksz, 128], tile index = c*ksz + dh.

    T[w_in, tile, w_out] = wdw[c, dh, w_in - w_out + pad] inside the band,
    else 0.  Partition-major so a wave DMA reads contiguous bytes per
    partition.  Optional per-channel scales applied before quantization.
    """
    if scales is not None:
        wdw = wdw * scales[:, None, None]
    pad = ksz // 2
    wi = np.arange(128)[:, None]
    wo = np.arange(128)[None, :]
    idx = wi - wo + pad
    valid = (idx >= 0) & (idx < ksz)
    idxc = np.clip(idx, 0, ksz - 1)
    T = wdw[:, :, idxc] * valid[None, None]          # [c, ksz, 128, 128]
    T = T.reshape(-1, 128, 128).transpose(1, 0, 2)   # [128, c*ksz, 128]
    return np.ascontiguousarray(T.astype(dtype))


def _split_excess_waits(nc, limit=1):
    """This container's walrus rejects >1 sync wait per instruction (and any
    wait on Drain beyond its own barrier). Hoist extras onto same-engine
    NoOps placed immediately before."""
    import bass_rust
    import concourse.mybir as mybir

    n_split = 0
    for fn in nc.m.functions:
        for bb in fn.blocks:
            insts = bb.instructions
            i = 0
            while i < len(insts):
                inst = insts[i]
                si = inst.sync_info
                lim = 0 if type(inst).__name__ == "InstDrain" else limit
                if si is not None and si.on_wait and len(si.on_wait) > lim:
                    waits = list(si.on_wait)
                    keep, extra = waits[:lim], waits[lim:]
                    pos = i
                    for j in range(0, len(extra), max(limit, 1)):
                        ch = extra[j : j + max(limit, 1)]
                        nop = mybir.InstNoOp(
                            name=f"waitsplit_{n_split}_{pos}",
                            engine=inst.engine,
                            ins=[],
                            outs=[],
                            sync_info=bass_rust.SyncInfo(on_wait=ch, on_update=[]),
                        )
                        insts.insert(pos, nop)
                        pos += 1
                        n_split += 1
                    inst.sync_info = bass_rust.SyncInfo(
                        on_wait=keep, on_update=list(si.on_update)
                    )
                    i = pos + 1
                else:
                    i += 1
    return n_split


def _build_program():
    import concourse.bass as bass
    import concourse.mybir as mybir
    import concourse.tile as tile

    F32 = mybir.dt.float32
    BF16 = mybir.dt.bfloat16
    F8 = mybir.dt.float8e4
    F8E3 = mybir.dt.float8e3
    AF = mybir.ActivationFunctionType
    OP = mybir.AluOpType

    nc = bass.Bass("TRN2", target_bir_lowering=False, debug=False, num_devices=8)

    # ---- DRAM parameters ----
    xin = nc.dram_tensor("x", [C, HW], BF16, kind="ExternalInput").ap()
    yin = nc.dram_tensor("y", [C, HW], BF16, kind="ExternalInput").ap()
    wq_d = nc.dram_tensor("wq", [256, C], BF16, kind="ExternalInput").ap()
    wkv_d = nc.dram_tensor("wkv", [256, 384], BF16, kind="ExternalInput").ap()
    wp_d = nc.dram_tensor("wp", [2, PC, C], BF16, kind="ExternalInput").ap()
    tq_d = nc.dram_tensor("tq", [128, C * 3, 128], F8, kind="ExternalInput").ap()
    tk_d = nc.dram_tensor("tk", [128, C * 7, 128], F8, kind="ExternalInput").ap()
    tv_d = nc.dram_tensor("tv", [128, C * 7, 128], F8E3, kind="ExternalInput").ap()
    idb_d = nc.dram_tensor("idb", [128, 128], BF16, kind="ExternalInput").ap()
    mask_d = nc.dram_tensor("maskbd", [PC, PC], F32, kind="ExternalInput").ap()
    temp_d = nc.dram_tensor("temprow", [1, C], F32, kind="ExternalInput").ap()
    svinv_d = nc.dram_tensor("svinv", [2, PC, 1], F32, kind="ExternalInput").ap()
    out_d = nc.dram_tensor("out", [C, HW], F32, kind="ExternalOutput").ap()

    with tile.TileContext(nc) as tc:
        import contextlib

        with contextlib.ExitStack() as ctx:
            consts = ctx.enter_context(tc.tile_pool(name="consts", bufs=1))
            s1 = ctx.enter_context(tc.tile_pool(name="s1", bufs=1))
            s2 = ctx.enter_context(tc.tile_pool(name="s2", bufs=1))
            s3 = ctx.enter_context(tc.tile_pool(name="s3", bufs=1))
            streams = ctx.enter_context(tc.tile_pool(name="streams", bufs=2))
            tpool8 = ctx.enter_context(tc.tile_pool(name="tpool8", bufs=4))
            tpoolb = ctx.enter_context(tc.tile_pool(name="tpoolb", bufs=3))
            scratch = ctx.enter_context(tc.tile_pool(name="scratch", bufs=2))
            sqpool = ctx.enter_context(tc.tile_pool(name="sqpool", bufs=1))
            ostage = ctx.enter_context(tc.tile_pool(name="ostage", bufs=2))
            misc = ctx.enter_context(tc.tile_pool(name="misc", bufs=2))
            stats = ctx.enter_context(tc.tile_pool(name="stats", bufs=1))
            vtpool = ctx.enter_context(tc.tile_pool(name="vtpool", bufs=1))

            # ---- load constants ----
            # second K-chunk padded to 128 rows (zeros) so every conv matmul
            # is K=128 — partial-row LDWEIGHTS serialize the PE pipeline
            wq0 = consts.tile([128, C], BF16)
            wq1 = consts.tile([128, C], BF16)
            nc.sync.dma_start(out=wq0, in_=wq_d[0:128, :])
            nc.sync.dma_start(out=wq1, in_=wq_d[128:256, :])
            wkv0 = consts.tile([128, 384], BF16)
            wkv1 = consts.tile([128, 384], BF16)
            nc.sync.dma_start(out=wkv0, in_=wkv_d[0:128, :])
            nc.sync.dma_start(out=wkv1, in_=wkv_d[128:256, :])
            wp0 = consts.tile([PC, C], BF16)
            wp1 = consts.tile([PC, C], BF16)
            nc.sync.dma_start(out=wp0, in_=wp_d[0])
            nc.sync.dma_start(out=wp1, in_=wp_d[1])
            identb = consts.tile([128, 128], BF16)
            nc.sync.dma_start(out=identb, in_=idb_d)
            maskbd = consts.tile([PC, PC], F32)
            nc.sync.dma_start(out=maskbd, in_=mask_d)
            temprow = consts.tile([1, C], F32)
            nc.sync.dma_start(out=temprow, in_=temp_d)
            svinv0 = consts.tile([PC, 1], F32)
            svinv1 = consts.tile([PC, 1], F32)
            nc.sync.dma_start(out=svinv0, in_=svinv_d[0])
            nc.sync.dma_start(out=svinv1, in_=svinv_d[1])
            onescol = consts.tile([128, 1], BF16)
            nc.vector.memset(onescol, 1.0)
            ones1 = consts.tile([1, 128], BF16)
            nc.vector.memset(ones1, 1.0)
            warm = consts.tile([128, 512], BF16)
            nc.vector.memset(warm, 0.5)

            # ---- big SBUF regions ----
            # channel-major: [w partitions, c*128 + h]; dw moving is contiguous
            bq = s1.tile([128, Himg * C], F8, tag="qv")
            bk = s2.tile([128, Himg * C + 32], F8, tag="kk")
            bv = s3.tile([128, Himg * C], BF16, tag="vv")
            bq3 = bq.rearrange("p (c h) -> p c h", h=Himg)
            bk3 = bk[:, 0 : Himg * C].rearrange("p (c h) -> p c h", h=Himg)
            bv3 = bv.rearrange("p (c h) -> p c h", h=Himg)
            # transposed views: [w, h, c] (strided in c)
            bq_hc = bq.rearrange("p (c h) -> p h c", h=Himg)
            bk_hc = bk[:, 0 : Himg * C].rearrange("p (c h) -> p h c", h=Himg)
            bv_hc = bv.rearrange("p (c h) -> p h c", h=Himg)

            # h-major staging rings for phase A
            HSTG, CHUNK = 32, 16
            stg = stats.tile([128, HSTG * 384], BF16, name="stg")
            stg_hc = stg.rearrange("p (h c) -> p h c", c=384)
            stg_ch = stg.rearrange("p (h c) -> p c h", c=384)
            HSTGQ = 16
            stgq = stats.tile([128, HSTGQ * C], BF16, name="stgq")
            stgq_hc = stgq.rearrange("p (h c) -> p h c", c=C)
            stgq_ch = stgq.rearrange("p (h c) -> p c h", c=C)

            partials = stats.tile([128, 2 * C], F32)
            partials_bf = stats.tile([128, 2 * C], BF16)

            # pair-0 v^T region for the first NB0 blocks; 128 partitions with
            # zero rows 96:128 so the fused output matmuls stay K=128
            vtb0 = vtpool.tile([128, NB0 * 512], BF16)
            nc.vector.memset(vtb0[PC:128, :], 0.0)

            def chan_ap(region3, c, col0, cnt):
                # [128, cnt] contiguous view: channel c, h col0..col0+cnt
                return region3[:, c, col0 : col0 + cnt]

            def copy_on(eng_idx, dst, src):
                if eng_idx == 0:
                    nc.vector.tensor_copy(dst, src)
                elif eng_idx == 1:
                    nc.scalar.activation(out=dst, in_=src, func=AF.Copy)
                else:
                    nc.gpsimd.tensor_copy(dst, src)

            # ================= Phase W+A: warmup + both 1x1 convs ===========
            with tc.tile_pool(name="psA_q", bufs=3, space="PSUM") as psA_q, \
                 tc.tile_pool(name="psA_kv", bufs=5, space="PSUM") as psA_kv:
                # warmup: dense matmul burst to lift the HAM clock gate while
                # the first input slabs are still in flight
                wps = psA_kv.tile([128, 512], F32, tag="ps", name="warmps")
                for i in range(12):
                    nc.tensor.matmul(
                        wps, warm[:, 0:128], warm, start=True, stop=True
                    )

                ptq, ptkv = [None], [None]
                for h in range(Himg):
                    sl = h % SLAB
                    if sl == 0:
                        qs0 = streams.tile([128, SLAB * 128], BF16, tag="qs0")
                        qs1 = streams.tile([128, SLAB * 128], BF16, tag="qs1")
                        nc.sync.dma_start(out=qs0, in_=xin[0:128, h * 128 : (h + SLAB) * 128])
                        nc.sync.dma_start(out=qs1[0:64], in_=xin[128:192, h * 128 : (h + SLAB) * 128])
                        nc.gpsimd.memset(qs1[64:128], 0.0)
                        ys0 = streams.tile([128, SLAB * 128], BF16, tag="ys0")
                        ys1 = streams.tile([128, SLAB * 128], BF16, tag="ys1")
                        nc.sync.dma_start(out=ys0, in_=yin[0:128, h * 128 : (h + SLAB) * 128])
                        nc.sync.dma_start(out=ys1[0:64], in_=yin[128:192, h * 128 : (h + SLAB) * 128])
                        nc.gpsimd.memset(ys1[64:128], 0.0)
                    # ---- q conv (2 rows per PSUM tile) ----
                    if h % 2 == 0:
                        ptq[0] = psA_q.tile([128, 2 * C], F32, tag="tp", name=f"ptq_{h}")
                    offq = (h % 2) * C
                    nc.tensor.matmul(
                        ptq[0][:, offq : offq + C],
                        qs0[:, sl * 128 : (sl + 1) * 128], wq0,
                        start=True, stop=False,
                    )
                    nc.tensor.matmul(
                        ptq[0][:, offq : offq + C],
                        qs1[:, sl * 128 : (sl + 1) * 128], wq1,
                        start=False, stop=True,
                    )
                    # ---- kv conv (1 row per PSUM tile) ----
                    ptkv[0] = psA_kv.tile([128, 384], F32, tag="ps", name=f"ptkv_{h}")
                    nc.tensor.matmul(
                        ptkv[0], ys0[:, sl * 128 : (sl + 1) * 128], wkv0,
                        start=True, stop=False,
                    )
                    nc.tensor.matmul(
                        ptkv[0], ys1[:, sl * 128 : (sl + 1) * 128], wkv1,
                        start=False, stop=True,
                    )
                    # ---- staged evictions, halves on DVE + ACT ----
                    hs = h % HSTG
                    nc.vector.tensor_copy(stg_hc[:, hs, 0:192], ptkv[0][:, 0:192])
                    nc.scalar.activation(
                        out=stg_hc[:, hs, 192:384], in_=ptkv[0][:, 192:384],
                        func=AF.Copy,
                    )
                    if h % 2 == 1:
                        qs_ = (h - 1) % HSTGQ
                        nc.vector.tensor_copy(stgq_hc[:, qs_, :], ptq[0][:, 0:C])
                        nc.scalar.activation(
                            out=stgq_hc[:, qs_ + 1, :], in_=ptq[0][:, C : 2 * C],
                            func=AF.Copy,
                        )
                    # ---- q repack every 8 rows (DVE/ACT halves) ----
                    if h % 8 == 7:
                        hq0 = h - 7
                        sq0 = hq0 % HSTGQ
                        eq = (h // 8) % 2
                        copy_on(eq, bq3[:, 0:96, hq0 : hq0 + 8], stgq_ch[:, 0:96, sq0 : sq0 + 8])
                        copy_on(1 - eq, bq3[:, 96:192, hq0 : hq0 + 8], stgq_ch[:, 96:192, sq0 : sq0 + 8])
                    # ---- kv repack every CHUNK rows: DVE 3, ACT 3, gpsimd 2
                    if h % CHUNK == CHUNK - 1:
                        hc0 = h - CHUNK + 1
                        s0 = hc0 % HSTG
                        pattern = [0, 1, 2, 0, 1, 0, 1, 2]
                        pi = 0
                        for reg3, c0, c1 in ((bk3, 0, C), (bv3, C, 2 * C)):
                            for qi in range(4):
                                cl = c0 + qi * 48
                                copy_on(
                                    pattern[pi],
                                    reg3[:, cl - c0 : cl - c0 + 48, hc0 : hc0 + CHUNK],
                                    stg_ch[:, cl : cl + 48, s0 : s0 + CHUNK],
                                )
                                pi += 1

            # ================= Phases B..G ==================================
            GB = 4                      # channels per PSUM bank group

            with tc.tile_pool(name="ps", bufs=4, space="PSUM") as ps, \
                 tc.tile_pool(name="pst", bufs=2, space="PSUM") as pst, \
                 tc.tile_pool(name="attnp", bufs=2, space="PSUM") as attnp_pool:

                def dw_phase(region3, t_dram, ksz, tdt, pool, tag, extra=None):
                    pad = ksz // 2
                    order = [pad] + [d for d in range(ksz) if d != pad]
                    cw = 4                      # channels per T-wave
                    wave_tile = [None]
                    pdw4 = [None]
                    for ci in range(C):
                        if ci % cw == 0:
                            nt = min(cw, C - ci) * ksz
                            wave_tile[0] = pool.tile(
                                [128, cw * ksz, 128], tdt, tag=tag,
                                name=f"tw_{tag}_{ci}",
                            )
                            i0 = ci * ksz
                            nc.sync.dma_start(
                                out=wave_tile[0][:, 0:nt, :],
                                in_=t_dram[:, i0 : i0 + nt, :],
                            )
                        tw = wave_tile[0]
                        if ci % GB == 0:
                            pdw4[0] = ps.tile([128, GB * 128], F32, tag="ps", name=f"pdw_{tag}_{ci}")
                        base = (ci % cw) * ksz
                        slot = (ci % GB) * 128
                        for j, dh in enumerate(order):
                            sh = dh - pad
                            cnt = Himg - abs(sh)
                            h0o, h0i = max(0, -sh), max(0, sh)
                            nc.tensor.matmul(
                                pdw4[0][:, slot + h0o : slot + h0o + cnt],
                                tw[:, base + dh, :],
                                chan_ap(region3, ci, h0i, cnt),
                                start=(j == 0),
                                stop=(j == len(order) - 1),
                            )
                        if ci % GB == GB - 1:
                            g0 = ci - (GB - 1)
                            # group writeback halves: DVE + ACT in parallel
                            half = GB * 128 // 2
                            nc.vector.tensor_copy(
                                region3[:, g0 : g0 + GB // 2, :], pdw4[0][:, 0:half]
                            )
                            nc.scalar.activation(
                                out=region3[:, g0 + GB // 2 : ci + 1, :],
                                in_=pdw4[0][:, half : 2 * half],
                                func=AF.Copy,
                            )
                            if extra is not None:
                                extra(ci // GB)

                def emit_sq(region3, g, sq_off, sq_eng):
                    """sum-of-squares for channel group g of region3 ->
                    partials[:, sq_off + 4g : ...]."""
                    g0 = g * GB
                    sc = sqpool.tile([128, GB * 128], BF16, tag=f"sq{sq_off}", name=f"sq_{sq_off}_{g}")
                    src = region3[:, g0 : g0 + GB, :]
                    if sq_eng == "gpsimd":
                        nc.gpsimd.tensor_tensor(sc, src, src, op=OP.mult)
                    else:
                        nc.scalar.activation(out=sc, in_=src, func=AF.Square)
                    sc3 = sc.rearrange("p (c h) -> p c h", h=Himg)
                    nc.vector.tensor_reduce(
                        partials[:, sq_off + g0 : sq_off + g0 + GB],
                        sc3,
                        axis=mybir.AxisListType.X,
                        op=OP.add,
                    )

                # ---- B1: q depthwise (pure matmul stream) ----
                dw_phase(bq3, tq_d, 3, F8, tpool8, "tw8")

                # ---- B2: k depthwise; q+k squares lag behind ----
                def k_extra(g):
                    emit_sq(bk3, g, C, "scalar")     # k squares on ACT
                    emit_sq(bq3, g, 0, "gpsimd")     # q squares on gpsimd
                dw_phase(bk3, tk_d, 7, F8, tpool8, "tw8", extra=k_extra)
                nc.vector.tensor_copy(partials_bf, partials)

                # ---- QK^T accumulators (filled during E) ----
                attnps = [
                    attnp_pool.tile([PC, PC], F32, tag="at", name=f"attnp_{P}")
                    for P in range(2)
                ]
                qk_jobs = [(P, h) for P in range(2) for h in range(Himg)]
                qk_i = [0]

                ezs = [None, None]
                mps = [None, None]
                sm_state = {}
                nb_done = [0]

                def softmax_piece1(P):
                    prow = pst.tile([1, PC], F32, tag="tp")
                    nc.tensor.matmul(
                        prow, onescol, partials_bf[:, PC * P : PC * P + PC],
                        start=True, stop=True,
                    )
                    sq_row = misc.tile([1, PC], F32, tag="m1")
                    nc.scalar.activation(out=sq_row, in_=prow, func=AF.Sqrt)
                    rq_row = misc.tile([1, PC], F32, tag="m2")
                    nc.vector.reciprocal(rq_row, sq_row)
                    nc.vector.tensor_tensor(
                        rq_row, rq_row, temprow[:, PC * P : PC * P + PC], op=OP.mult
                    )
                    rq_bf = misc.tile([1, PC], BF16, tag="m3")
                    nc.vector.tensor_copy(rq_bf, rq_row)
                    pcol = pst.tile([PC, 1], F32, tag="tp")
                    nc.tensor.matmul(
                        pcol, partials_bf[:, C + PC * P : C + PC * P + PC], onescol,
                        start=True, stop=True,
                    )
                    sq_col = misc.tile([PC, 1], F32, tag="m4")
                    nc.scalar.activation(out=sq_col, in_=pcol, func=AF.Sqrt)
                    rk_col = misc.tile([PC, 1], F32, tag="m5")
                    nc.vector.reciprocal(rk_col, sq_col)
                    return rq_bf, rk_col

                def softmax_piece2(P, rq_bf, rk_col):
                    attnp = attnps[P]
                    prep = pst.tile([PC, PC], F32, tag="tp")
                    nc.tensor.matmul(
                        prep, ones1[:, 0:PC], rq_bf, start=True, stop=True
                    )
                    rqrep = misc.tile([PC, PC], F32, tag="m6")
                    nc.vector.tensor_copy(rqrep, prep)
                    t1 = misc.tile([PC, PC], F32, tag="m7")
                    nc.vector.tensor_tensor(t1, attnp, rqrep, op=OP.mult)
                    e1 = misc.tile([PC, PC], F32, tag="m8")
                    nc.scalar.activation(out=e1, in_=t1, func=AF.Exp, scale=rk_col)
                    ezero = stats.tile([PC, 128], BF16, tag=f"ez{P}")
                    nc.vector.memset(ezero[:, PC:128], 0.0)
                    nc.vector.tensor_tensor(ezero[:, 0:PC], e1, maskbd, op=OP.mult)
                    pcs = pst.tile([PC, 1], F32, tag="tp")
                    nc.tensor.matmul(
                        pcs, ezero[:, 0:PC], onescol[0:PC], start=True, stop=True
                    )
                    recip = stats.tile([PC, 1], F32, tag=f"rc{P}")
                    nc.vector.reciprocal(recip, pcs)
                    ezs[P] = (ezero, recip)

                def mps_prep(P):
                    ezero, recip = ezs[P]
                    ezt_ps = pst.tile([PC, PC], BF16, tag="tp")
                    nc.tensor.transpose(ezt_ps, ezero[:, 0:PC], identb[0:PC, 0:PC])
                    ezt = misc.tile([PC, PC], BF16, tag="m9")
                    nc.vector.tensor_copy(ezt, ezt_ps)
                    wsc = misc.tile([PC, C], BF16, tag="m10")
                    nc.vector.tensor_scalar_mul(wsc, (wp0, wp1)[P], recip)
                    pmp = ps.tile([PC, C], F32, tag="ps")
                    nc.tensor.matmul(pmp, ezt, wsc, start=True, stop=True)
                    # [128, 256] zero-padded so the fused output matmuls are
                    # full-array K=128 / M=128 (no row/col-group masks)
                    mp = stats.tile([128, 256], BF16, tag=f"mp{P}")
                    nc.vector.memset(mp, 0.0)
                    # descale the fp8e3m4-scaled v channels (partition dim = d)
                    nc.vector.tensor_scalar_mul(mp[0:PC, 0:C], pmp, (svinv0, svinv1)[P])
                    mps[P] = mp

                def emit_vtb0(nb):
                    ptv = pst.tile([PC, 512], BF16, tag="tp", name=f"ptv0_{nb}")
                    for hh in range(4):
                        nc.tensor.transpose(
                            ptv[:, hh * 128 : (hh + 1) * 128],
                            bv_hc[:, nb * 4 + hh, 0:PC],
                            identb,
                        )
                    nc.vector.tensor_copy(
                        vtb0[0:PC, nb * 512 : (nb + 1) * 512], ptv
                    )

                def v_extra(g):
                    # sprinkle QK^T matmuls through the dense v-dw stream
                    # (front-loaded: done before the vtb0 transposes pile on)
                    end = min(len(qk_jobs), (g + 1) * 11)
                    while qk_i[0] < end:
                        P, h = qk_jobs[qk_i[0]]
                        nc.tensor.matmul(
                            attnps[P],
                            bk_hc[:, h, PC * P : PC * P + PC],
                            bq_hc[:, h, PC * P : PC * P + PC],
                            start=(h == 0),
                            stop=(h == Himg - 1),
                        )
                        qk_i[0] += 1
                    if g == 24:
                        sm_state[0] = softmax_piece1(0)
                    elif g == 25:
                        softmax_piece2(0, *sm_state[0])
                        sm_state[1] = softmax_piece1(1)
                    elif g == 26:
                        softmax_piece2(1, *sm_state[1])
                    elif g == 27:
                        mps_prep(0)
                    elif g == 28:
                        mps_prep(1)
                    if g >= 24:
                        j = g - 24 + 1          # 1..24
                        target = j * NB0 // 24
                        while nb_done[0] < target:
                            emit_vtb0(nb_done[0])
                            nb_done[0] += 1

                # ---- E: v depthwise with everything interleaved ----
                dw_phase(bv3, tv_d, 7, F8E3, tpoolb, "twb", extra=v_extra)
                while nb_done[0] < NB0:
                    emit_vtb0(nb_done[0])
                    nb_done[0] += 1

                # ---- G tail: remaining transposes + fused (attn@v)+proj ----
                # pre-zero the pad rows of the scratch ring buffers once; the
                # in-loop copies only touch rows 0:PC, so the zeros persist
                for i in range(2):
                    z1 = scratch.tile([128, 512], BF16, tag="vtb1", name=f"z1_{i}")
                    nc.gpsimd.memset(z1[PC:128, :], 0.0)
                    z0 = scratch.tile([128, 512], BF16, tag="vtb0b", name=f"z0_{i}")
                    nc.gpsimd.memset(z0[PC:128, :], 0.0)
                for nb in range(Himg // 4):
                    h0 = nb * 4
                    ptv = pst.tile([PC, 512], BF16, tag="tp", name=f"ptv1_{nb}")
                    for hh in range(4):
                        nc.tensor.transpose(
                            ptv[:, hh * 128 : (hh + 1) * 128],
                            bv_hc[:, h0 + hh, PC : PC + PC],
                            identb,
                        )
                    vtb1 = scratch.tile([128, 512], BF16, tag="vtb1", name=f"vtb1_{nb}")
                    nc.vector.tensor_copy(vtb1[0:PC, :], ptv)
                    if nb >= NB0:
                        ptv0 = pst.tile([PC, 512], BF16, tag="tp", name=f"ptv0b_{nb}")
                        for hh in range(4):
                            nc.tensor.transpose(
                                ptv0[:, hh * 128 : (hh + 1) * 128],
                                bv_hc[:, h0 + hh, 0:PC],
                                identb,
                            )
                        vtb0b = scratch.tile([128, 512], BF16, tag="vtb0b", name=f"vtb0b_{nb}")
                        nc.scalar.activation(out=vtb0b[0:PC, :], in_=ptv0, func=AF.Copy)
                        vsrc0 = vtb0b
                    else:
                        vsrc0 = vtb0[:, nb * 512 : (nb + 1) * 512]
                    n = nb * 512
                    for mi, (r0, r1) in enumerate(((0, 128), (128, 192))):
                        mw = r1 - r0
                        po = ps.tile([128, 512], F32, tag="ps", name=f"po_{mi}_{nb}")
                        nc.tensor.matmul(
                            po, mps[0][:, mi * 128 : mi * 128 + 128], vsrc0,
                            start=True, stop=False,
                        )
                        nc.tensor.matmul(
                            po, mps[1][:, mi * 128 : mi * 128 + 128], vtb1,
                            start=False, stop=True,
                        )
                        so = ostage.tile([mw, 512], F32, tag="os", name=f"so_{mi}_{nb}")
                        copy_on(mi, so, po[0:mw, :])
                        nc.sync.dma_start(out=out_d[r0:r1, n : n + 512], in_=so)

    _split_excess_waits(nc)
    return nc


def _get_program():
    global _PROG
    if _PROG is None:
        _PROG = _build_program()
    return _PROG


def kernel(x, y, q_w, q_dw_w, kv_w, kv_dw_w, proj_w, temperature):
    return _run(x, y, q_w, q_dw_w, kv_w, kv_dw_w, proj_w, temperature)[0]


def _run(x, y, q_w, q_dw_w, kv_w, kv_dw_w, proj_w, temperature, trace=False):
    from concourse.bass_utils import run_bass_kernel_spmd

    x = np.asarray(x, dtype=np.float32).reshape(B, C, HW).astype(ml_dtypes.bfloat16)
    y = np.asarray(y, dtype=np.float32).reshape(B, C, HW).astype(ml_dtypes.bfloat16)
    q_w = np.asarray(q_w, dtype=np.float32)
    kv_w = np.asarray(kv_w, dtype=np.float32)
    proj_w = np.asarray(proj_w, dtype=np.float32)
    q_dw_w = np.asarray(q_dw_w, dtype=np.float32)
    kv_dw_w = np.asarray(kv_dw_w, dtype=np.float32)
    temperature = np.asarray(temperature, dtype=np.float32).reshape(HEADS)

    # conv weights padded to 256 K-rows (zeros) for K=128-homogeneous matmuls
    wq = np.zeros((256, C), np.float32)
    wq[0:C] = q_w[:, :, 0, 0].T
    wq = wq.astype(ml_dtypes.bfloat16)
    wkv = np.zeros((256, 384), np.float32)
    wkv[0:C] = kv_w[:, :, 0, 0].T
    wkv = wkv.astype(ml_dtypes.bfloat16)
    # v depthwise in fp8e3m4 with per-channel power-of-2 scaling; inverse
    # scales ride a [PC,1] per-partition multiply on the fused projection
    wv = kv_dw_w[C : 2 * C, 0]                              # [C, 7, 7]
    v_absmax = np.abs(wv).reshape(C, -1).max(axis=1)
    v_scale = 2.0 ** np.floor(np.log2(14.0 / v_absmax))
    svinv = (1.0 / v_scale).astype(np.float32).reshape(2, PC, 1)
    wpT = proj_w[:, :, 0, 0].T                              # [c_in, c_out]
    wp = np.stack([wpT[0:PC], wpT[PC:C]]).astype(ml_dtypes.bfloat16)
    tq = _build_toeplitz(q_dw_w[:, 0], 3, ml_dtypes.float8_e4m3)
    tk = _build_toeplitz(kv_dw_w[0:C, 0], 7, ml_dtypes.float8_e4m3)
    tv = _build_toeplitz(wv, 7, ml_dtypes.float8_e3m4, scales=v_scale)
    idb = np.eye(128, dtype=ml_dtypes.bfloat16)
    maskbd = np.zeros((PC, PC), np.float32)
    maskbd[0:DHC, 0:DHC] = 1.0
    maskbd[DHC:PC, DHC:PC] = 1.0
    temprow = np.repeat(temperature, DHC).reshape(1, C)

    shared = {
        "wq": wq, "wkv": wkv, "wp": wp, "tq": tq, "tk": tk, "tv": tv,
        "idb": idb, "maskbd": maskbd, "temprow": temprow, "svinv": svinv,
    }
    in_maps = [dict(shared, x=x[i], y=y[i]) for i in range(B)]

    nc = _get_program()
    res = run_bass_kernel_spmd(
        nc, in_maps, core_ids=list(range(B)), trace=trace
    )
    out = np.stack([res.results[i]["out"] for i in range(B)])
    return out.reshape(B, C, Himg, Wimg).astype(np.float32), res


# revision 30
# speedup vs baseline: 1.1137x; 1.0281x over previous
"""nn_CrossAttention Trainium2 kernel — 8-core data-parallel over batch.

Per core (batch slice b=1):
  Warmup matmul burst un-throttles the PE clock (HAM) while the first
  input slabs stream in.  Both 1x1 convs run interleaved in bf16
  (stationary = 8-row input slabs); h-major PSUM tiles are staged
  contiguously (eviction halves split across DVE/ACT, 3+5 PSUM banks
  deep) then repacked to channel-major SBUF regions by DVE/ACT/gpsimd.
  Depthwise 3x3/7x7 as per-(channel, dh) banded-Toeplitz matmuls on the
  TensorEngine (q/k tables fp8e4m3, v tables fp8e3m4 with per-channel
  power-of-2 scaling descaled via a per-partition multiply on the fused
  projection matrix).  Sum-of-squares for the l2 norms lag behind the
  k-dw loop (q squares on gpsimd, k squares on ACT, reduces on DVE).
  QK^T matmuls for both head-pairs are sprinkled through the v-dw
  matmul stream (their strided moving operands are slow, so they ride
  the dense stream instead of running alone and cold); softmax chains
  and half the pair-0 v^T transposes also interleave with v-dw.  The
  output tail fuses the remaining transposes with the (attn@v)+proj
  matmuls, with one warm-keeper matmul per block so the clock gate
  stays open.
"""

import sys

sys.path.insert(0, "/opt/trn_rl_repo")

import numpy as np
import ml_dtypes

B, C, Himg, Wimg = 8, 192, 128, 128
HW = Himg * Wimg
HEADS, DHC = 4, 48      # heads, channels per head
PC = 96                 # channels per head-pair
SLAB = 8                # image rows per input stream slab
NB0 = 16                # pair-0 v^T blocks transposed during v-dw

_PROG = None            # cached (nc, meta)


def _build_toeplitz(wdw, ksz, dtype, scales=None):
    """wdw [c, ksz, ksz] f32 -> [128, c*ksz, 128], tile index = c*ksz + dh.

    T[w_in, tile, w_out] = wdw[c, dh, w_in - w_out + pad] inside the band,
    else 0.  Partition-major so a wave DMA reads contiguous bytes per
    partition.  Optional per-channel scales applied before quantization.
    """
    if scales is not None:
        wdw = wdw * scales[:, None, None]
    pad = ksz // 2
    wi = np.arange(128)[:, None]
    wo = np.arange(128)[None, :]
    idx = wi - wo + pad
    valid = (idx >= 0) & (idx < ksz)
    idxc = np.clip(idx, 0, ksz - 1)
    T = wdw[:, :, idxc] * valid[None, None]          # [c, ksz, 128, 128]
    T = T.reshape(-1, 128, 128).transpose(1, 0, 2)   # [128, c*ksz, 128]
    return np.ascontiguousarray(T.astype(dtype))


def _split_excess_waits(nc, limit=1):
    """This container's walrus rejects >1 sync wait per instruction (and any
    wait on Drain beyond its own barrier). Hoist extras onto same-engine
    NoOps placed immediately before."""
    import bass_rust
    import concourse.mybir as mybir

    n_split = 0
    for fn in nc.m.functions:
        for bb in fn.blocks:
            insts = bb.instructions
            i = 0
            while i < len(insts):
                inst = insts[i]
                si = inst.sync_info
                lim = 0 if type(inst).__name__ == "InstDrain" else limit
                if si is not None and si.on_wait and len(si.on_wait) > lim:
                    waits = list(si.on_wait)
                    keep, extra = waits[:lim], waits[lim:]
                    pos = i
                    for j in range(0, len(extra), max(limit, 1)):
                        ch = extra[j : j + max(limit, 1)]
                        nop = mybir.InstNoOp(
                            name=f"waitsplit_{n_split}_{pos}",
                            engine=inst.engine,
                            ins=[],
                            outs=[],
                            sync_info=bass_rust.SyncInfo(on_wait=ch, on_update=[]),
                        )
                        insts.insert(pos, nop)
                        pos += 1
                        n_split += 1
                    inst.sync_info = bass_rust.SyncInfo(
                        on_wait=keep, on_update=list(si.on_update)
                    )
                    i = pos + 1
                else:
                    i += 1
    return n_split


def _build_program():
    import concourse.bass as bass
    import concourse.mybir as mybir
    import concourse.tile as tile

    F32 = mybir.dt.float32
    BF16 = mybir.dt.bfloat16
    F8 = mybir.dt.float8e4
    F8E3 = mybir.dt.float8e3
    AF = mybir.ActivationFunctionType
    OP = mybir.AluOpType

    nc = bass.Bass("TRN2", target_bir_lowering=False, debug=False, num_devices=8)

    # ---- DRAM parameters ----
    xin = nc.dram_tensor("x", [C, HW], BF16, kind="ExternalInput").ap()
    yin = nc.dram_tensor("y", [C, HW], BF16, kind="ExternalInput").ap()
    wq_d = nc.dram_tensor("wq", [256, C], BF16, kind="ExternalInput").ap()
    wkv_d = nc.dram_tensor("wkv", [256, 384], BF16, kind="ExternalInput").ap()
    wp_d = nc.dram_tensor("wp", [2, PC, C], BF16, kind="ExternalInput").ap()
    tq_d = nc.dram_tensor("tq", [128, C * 3, 128], F8, kind="ExternalInput").ap()
    tk_d = nc.dram_tensor("tk", [128, C * 7, 128], F8, kind="ExternalInput").ap()
    tv_d = nc.dram_tensor("tv", [128, C * 7, 128], F8E3, kind="ExternalInput").ap()
    idb_d = nc.dram_tensor("idb", [128, 128], BF16, kind="ExternalInput").ap()
    mask_d = nc.dram_tensor("maskbd", [PC, PC], F32, kind="ExternalInput").ap()
    temp_d = nc.dram_tensor("temprow", [1, C], F32, kind="ExternalInput").ap()
    svinv_d = nc.dram_tensor("svinv", [2, PC, 1], F32, kind="ExternalInput").ap()
    out_d = nc.dram_tensor("out", [C, HW], F32, kind="ExternalOutput").ap()

    with tile.TileContext(nc) as tc:
        import contextlib

        with contextlib.ExitStack() as ctx:
            consts = ctx.enter_context(tc.tile_pool(name="consts", bufs=1))
            s1 = ctx.enter_context(tc.tile_pool(name="s1", bufs=1))
            s2 = ctx.enter_context(tc.tile_pool(name="s2", bufs=1))
            s3 = ctx.enter_context(tc.tile_pool(name="s3", bufs=1))
            streams = ctx.enter_context(tc.tile_pool(name="streams", bufs=2))
            tpool8 = ctx.enter_context(tc.tile_pool(name="tpool8", bufs=4))
            tpoolb = ctx.enter_context(tc.tile_pool(name="tpoolb", bufs=3))
            scratch = ctx.enter_context(tc.tile_pool(name="scratch", bufs=2))
            sqpool = ctx.enter_context(tc.tile_pool(name="sqpool", bufs=1))
            ostage = ctx.enter_context(tc.tile_pool(name="ostage", bufs=2))
            misc = ctx.enter_context(tc.tile_pool(name="misc", bufs=2))
            stats = ctx.enter_context(tc.tile_pool(name="stats", bufs=1))
            vtpool = ctx.enter_context(tc.tile_pool(name="vtpool", bufs=1))

            # ---- load constants ----
            # second K-chunk padded to 128 rows (zeros) so every conv matmul
            # is K=128 — partial-row LDWEIGHTS serialize the PE pipeline
            wq0 = consts.tile([128, C], BF16)
            wq1 = consts.tile([128, C], BF16)
            nc.sync.dma_start(out=wq0, in_=wq_d[0:128, :])
            nc.sync.dma_start(out=wq1, in_=wq_d[128:256, :])
            wkv0 = consts.tile([128, 384], BF16)
            wkv1 = consts.tile([128, 384], BF16)
            nc.sync.dma_start(out=wkv0, in_=wkv_d[0:128, :])
            nc.sync.dma_start(out=wkv1, in_=wkv_d[128:256, :])
            wp0 = consts.tile([PC, C], BF16)
            wp1 = consts.tile([PC, C], BF16)
            nc.sync.dma_start(out=wp0, in_=wp_d[0])
            nc.sync.dma_start(out=wp1, in_=wp_d[1])
            identb = consts.tile([128, 128], BF16)
            nc.sync.dma_start(out=identb, in_=idb_d)
            maskbd = consts.tile([PC, PC], F32)
            nc.sync.dma_start(out=maskbd, in_=mask_d)
            temprow = consts.tile([1, C], F32)
            nc.sync.dma_start(out=temprow, in_=temp_d)
            svinv0 = consts.tile([PC, 1], F32)
            svinv1 = consts.tile([PC, 1], F32)
            nc.sync.dma_start(out=svinv0, in_=svinv_d[0])
            nc.sync.dma_start(out=svinv1, in_=svinv_d[1])
            onescol = consts.tile([128, 1], BF16)
            nc.vector.memset(onescol, 1.0)
            ones1 = consts.tile([1, 128], BF16)
            nc.vector.memset(ones1, 1.0)
            warm = consts.tile([128, 512], BF16)
            nc.vector.memset(warm, 0.5)

            # ---- big SBUF regions ----
            # channel-major: [w partitions, c*128 + h]; dw moving is contiguous
            bq = s1.tile([128, Himg * C], F8, tag="qv")
            bk = s2.tile([128, Himg * C + 32], F8, tag="kk")
            bv = s3.tile([128, Himg * C], BF16, tag="vv")
            bq3 = bq.rearrange("p (c h) -> p c h", h=Himg)
            bk3 = bk[:, 0 : Himg * C].rearrange("p (c h) -> p c h", h=Himg)
            bv3 = bv.rearrange("p (c h) -> p c h", h=Himg)
            # transposed views: [w, h, c] (strided in c)
            bq_hc = bq.rearrange("p (c h) -> p h c", h=Himg)
            bk_hc = bk[:, 0 : Himg * C].rearrange("p (c h) -> p h c", h=Himg)
            bv_hc = bv.rearrange("p (c h) -> p h c", h=Himg)

            # h-major staging rings for phase A
            HSTG, CHUNK = 32, 16
            stg = stats.tile([128, HSTG * 384], BF16, name="stg")
            stg_hc = stg.rearrange("p (h c) -> p h c", c=384)
            stg_ch = stg.rearrange("p (h c) -> p c h", c=384)
            HSTGQ = 16
            stgq = stats.tile([128, HSTGQ * C], BF16, name="stgq")
            stgq_hc = stgq.rearrange("p (h c) -> p h c", c=C)
            stgq_ch = stgq.rearrange("p (h c) -> p c h", c=C)

            partials = stats.tile([128, 2 * C], F32)
            partials_bf = stats.tile([128, 2 * C], BF16)

            # pair-0 v^T region for the first NB0 blocks; 128 partitions with
            # zero rows 96:128 so the fused output matmuls stay K=128
            vtb0 = vtpool.tile([128, NB0 * 512], BF16)
            nc.vector.memset(vtb0[PC:128, :], 0.0)

            def chan_ap(region3, c, col0, cnt):
                # [128, cnt] contiguous view: channel c, h col0..col0+cnt
                return region3[:, c, col0 : col0 + cnt]

            def copy_on(eng_idx, dst, src):
                if eng_idx == 0:
                    nc.vector.tensor_copy(dst, src)
                elif eng_idx == 1:
                    nc.scalar.activation(out=dst, in_=src, func=AF.Copy)
                else:
                    nc.gpsimd.tensor_copy(dst, src)

            # ================= Phase W+A: warmup + both 1x1 convs ===========
            with tc.tile_pool(name="psA_q", bufs=3, space="PSUM") as psA_q, \
                 tc.tile_pool(name="psA_kv", bufs=5, space="PSUM") as psA_kv:
                # warmup: dense matmul burst to lift the HAM clock gate while
                # the first input slabs are still in flight
                wps = psA_kv.tile([128, 512], F32, tag="ps", name="warmps")
                for i in range(12):
                    nc.tensor.matmul(
                        wps, warm[:, 0:128], warm, start=True, stop=True
                    )

                ptq, ptkv = [None], [None]
                for h in range(Himg):
                    sl = h % SLAB
                    if sl == 0:
                        qs0 = streams.tile([128, SLAB * 128], BF16, tag="qs0")
                        qs1 = streams.tile([128, SLAB * 128], BF16, tag="qs1")
                        nc.sync.dma_start(out=qs0, in_=xin[0:128, h * 128 : (h + SLAB) * 128])
                        nc.sync.dma_start(out=qs1[0:64], in_=xin[128:192, h * 128 : (h + SLAB) * 128])
                        nc.gpsimd.memset(qs1[64:128], 0.0)
                        ys0 = streams.tile([128, SLAB * 128], BF16, tag="ys0")
                        ys1 = streams.tile([128, SLAB * 128], BF16, tag="ys1")
                        nc.sync.dma_start(out=ys0, in_=yin[0:128, h * 128 : (h + SLAB) * 128])
                        nc.sync.dma_start(out=ys1[0:64], in_=yin[128:192, h * 128 : (h + SLAB) * 128])
                        nc.gpsimd.memset(ys1[64:128], 0.0)
                    # ---- q conv (2 rows per PSUM tile) ----
                    if h % 2 == 0:
                        ptq[0] = psA_q.tile([128, 2 * C], F32, tag="tp", name=f"ptq_{h}")
                    offq = (h % 2) * C
                    nc.tensor.matmul(
                        ptq[0][:, offq : offq + C],
                        qs0[:, sl * 128 : (sl + 1) * 128], wq0,
                        start=True, stop=False,
                    )
                    nc.tensor.matmul(
                        ptq[0][:, offq : offq + C],
                        qs1[:, sl * 128 : (sl + 1) * 128], wq1,
                        start=False, stop=True,
                    )
                    # ---- kv conv (1 row per PSUM tile) ----
                    ptkv[0] = psA_kv.tile([128, 384], F32, tag="ps", name=f"ptkv_{h}")
                    nc.tensor.matmul(
                        ptkv[0], ys0[:, sl * 128 : (sl + 1) * 128], wkv0,
                        start=True, stop=False,
                    )
                    nc.tensor.matmul(
                        ptkv[0], ys1[:, sl * 128 : (sl + 1) * 128], wkv1,
                        start=False, stop=True,
                    )
                    # ---- staged evictions, halves on DVE + ACT ----
                    hs = h % HSTG
                    nc.vector.tensor_copy(stg_hc[:, hs, 0:192], ptkv[0][:, 0:192])
                    nc.scalar.activation(
                        out=stg_hc[:, hs, 192:384], in_=ptkv[0][:, 192:384],
                        func=AF.Copy,
                    )
                    if h % 2 == 1:
                        qs_ = (h - 1) % HSTGQ
                        nc.vector.tensor_copy(stgq_hc[:, qs_, :], ptq[0][:, 0:C])
                        nc.scalar.activation(
                            out=stgq_hc[:, qs_ + 1, :], in_=ptq[0][:, C : 2 * C],
                            func=AF.Copy,
                        )
                    # ---- q repack every 8 rows (DVE/ACT halves) ----
                    if h % 8 == 7:
                        hq0 = h - 7
                        sq0 = hq0 % HSTGQ
                        eq = (h // 8) % 2
                        copy_on(eq, bq3[:, 0:96, hq0 : hq0 + 8], stgq_ch[:, 0:96, sq0 : sq0 + 8])
                        copy_on(1 - eq, bq3[:, 96:192, hq0 : hq0 + 8], stgq_ch[:, 96:192, sq0 : sq0 + 8])
                    # ---- kv repack every CHUNK rows: DVE 3, ACT 3, gpsimd 2
                    if h % CHUNK == CHUNK - 1:
                        hc0 = h - CHUNK + 1
                        s0 = hc0 % HSTG
                        pattern = [0, 1, 2, 0, 2, 1, 0, 2]
                        pi = 0
                        for reg3, c0, c1 in ((bk3, 0, C), (bv3, C, 2 * C)):
                            for qi in range(4):
                                cl = c0 + qi * 48
                                copy_on(
                                    pattern[pi],
                                    reg3[:, cl - c0 : cl - c0 + 48, hc0 : hc0 + CHUNK],
                                    stg_ch[:, cl : cl + 48, s0 : s0 + CHUNK],
                                )
                                pi += 1

            # ================= Phases B..G ==================================
            GB = 4                      # channels per PSUM bank group

            with tc.tile_pool(name="ps", bufs=4, space="PSUM") as ps, \
                 tc.tile_pool(name="pst", bufs=2, space="PSUM") as pst, \
                 tc.tile_pool(name="attnp", bufs=2, space="PSUM") as attnp_pool:

                def dw_phase(region3, t_dram, ksz, tdt, pool, tag, extra=None):
                    pad = ksz // 2
                    order = [pad] + [d for d in range(ksz) if d != pad]
                    cw = 4                      # channels per T-wave
                    wave_tile = [None]
                    pdw4 = [None]
                    for ci in range(C):
                        if ci % cw == 0:
                            nt = min(cw, C - ci) * ksz
                            wave_tile[0] = pool.tile(
                                [128, cw * ksz, 128], tdt, tag=tag,
                                name=f"tw_{tag}_{ci}",
                            )
                            i0 = ci * ksz
                            nc.sync.dma_start(
                                out=wave_tile[0][:, 0:nt, :],
                                in_=t_dram[:, i0 : i0 + nt, :],
                            )
                        tw = wave_tile[0]
                        if ci % GB == 0:
                            pdw4[0] = ps.tile([128, GB * 128], F32, tag="ps", name=f"pdw_{tag}_{ci}")
                        base = (ci % cw) * ksz
                        slot = (ci % GB) * 128
                        for j, dh in enumerate(order):
                            sh = dh - pad
                            cnt = Himg - abs(sh)
                            h0o, h0i = max(0, -sh), max(0, sh)
                            nc.tensor.matmul(
                                pdw4[0][:, slot + h0o : slot + h0o + cnt],
                                tw[:, base + dh, :],
                                chan_ap(region3, ci, h0i, cnt),
                                start=(j == 0),
                                stop=(j == len(order) - 1),
                            )
                        if ci % GB == GB - 1:
                            g0 = ci - (GB - 1)
                            # group writeback halves: DVE + ACT in parallel
                            half = GB * 128 // 2
                            nc.vector.tensor_copy(
                                region3[:, g0 : g0 + GB // 2, :], pdw4[0][:, 0:half]
                            )
                            nc.scalar.activation(
                                out=region3[:, g0 + GB // 2 : ci + 1, :],
                                in_=pdw4[0][:, half : 2 * half],
                                func=AF.Copy,
                            )
                            if extra is not None:
                                extra(ci // GB)

                def emit_sq(region3, g, sq_off, sq_eng):
                    """sum-of-squares for channel group g of region3 ->
                    partials[:, sq_off + 4g : ...]."""
                    g0 = g * GB
                    sc = sqpool.tile([128, GB * 128], BF16, tag=f"sq{sq_off}", name=f"sq_{sq_off}_{g}")
                    src = region3[:, g0 : g0 + GB, :]
                    if sq_eng == "gpsimd":
                        nc.gpsimd.tensor_tensor(sc, src, src, op=OP.mult)
                    else:
                        nc.scalar.activation(out=sc, in_=src, func=AF.Square)
                    sc3 = sc.rearrange("p (c h) -> p c h", h=Himg)
                    nc.vector.tensor_reduce(
                        partials[:, sq_off + g0 : sq_off + g0 + GB],
                        sc3,
                        axis=mybir.AxisListType.X,
                        op=OP.add,
                    )

                # ---- B1: q depthwise (pure matmul stream) ----
                dw_phase(bq3, tq_d, 3, F8, tpool8, "tw8")

                # ---- B2: k depthwise; q+k squares lag behind ----
                def k_extra(g):
                    emit_sq(bk3, g, C, "scalar")     # k squares on ACT
                    emit_sq(bq3, g, 0, "gpsimd")     # q squares on gpsimd
                dw_phase(bk3, tk_d, 7, F8, tpool8, "tw8", extra=k_extra)
                nc.vector.tensor_copy(partials_bf, partials)

                # ---- QK^T accumulators (filled during E) ----
                attnps = [
                    attnp_pool.tile([PC, PC], F32, tag="at", name=f"attnp_{P}")
                    for P in range(2)
                ]
                qk_jobs = [(P, h) for P in range(2) for h in range(Himg)]
                qk_i = [0]

                ezs = [None, None]
                mps = [None, None]
                sm_state = {}
                nb_done = [0]

                def softmax_piece1(P):
                    prow = pst.tile([1, PC], F32, tag="tp")
                    nc.tensor.matmul(
                        prow, onescol, partials_bf[:, PC * P : PC * P + PC],
                        start=True, stop=True,
                    )
                    sq_row = misc.tile([1, PC], F32, tag="m1")
                    nc.scalar.activation(out=sq_row, in_=prow, func=AF.Sqrt)
                    rq_row = misc.tile([1, PC], F32, tag="m2")
                    nc.vector.reciprocal(rq_row, sq_row)
                    nc.vector.tensor_tensor(
                        rq_row, rq_row, temprow[:, PC * P : PC * P + PC], op=OP.mult
                    )
                    rq_bf = misc.tile([1, PC], BF16, tag="m3")
                    nc.vector.tensor_copy(rq_bf, rq_row)
                    pcol = pst.tile([PC, 1], F32, tag="tp")
                    nc.tensor.matmul(
                        pcol, partials_bf[:, C + PC * P : C + PC * P + PC], onescol,
                        start=True, stop=True,
                    )
                    sq_col = misc.tile([PC, 1], F32, tag="m4")
                    nc.scalar.activation(out=sq_col, in_=pcol, func=AF.Sqrt)
                    rk_col = misc.tile([PC, 1], F32, tag="m5")
                    nc.vector.reciprocal(rk_col, sq_col)
                    return rq_bf, rk_col

                def softmax_piece2(P, rq_bf, rk_col):
                    attnp = attnps[P]
                    prep = pst.tile([PC, PC], F32, tag="tp")
                    nc.tensor.matmul(
                        prep, ones1[:, 0:PC], rq_bf, start=True, stop=True
                    )
                    rqrep = misc.tile([PC, PC], F32, tag="m6")
                    nc.vector.tensor_copy(rqrep, prep)
                    t1 = misc.tile([PC, PC], F32, tag="m7")
                    nc.vector.tensor_tensor(t1, attnp, rqrep, op=OP.mult)
                    e1 = misc.tile([PC, PC], F32, tag="m8")
                    nc.scalar.activation(out=e1, in_=t1, func=AF.Exp, scale=rk_col)
                    ezero = stats.tile([PC, 128], BF16, tag=f"ez{P}")
                    nc.vector.memset(ezero[:, PC:128], 0.0)
                    nc.vector.tensor_tensor(ezero[:, 0:PC], e1, maskbd, op=OP.mult)
                    pcs = pst.tile([PC, 1], F32, tag="tp")
                    nc.tensor.matmul(
                        pcs, ezero[:, 0:PC], onescol[0:PC], start=True, stop=True
                    )
                    recip = stats.tile([PC, 1], F32, tag=f"rc{P}")
                    nc.vector.reciprocal(recip, pcs)
                    ezs[P] = (ezero, recip)

                def mps_prep(P):
                    ezero, recip = ezs[P]
                    ezt_ps = pst.tile([PC, PC], BF16, tag="tp")
                    nc.tensor.transpose(ezt_ps, ezero[:, 0:PC], identb[0:PC, 0:PC])
                    ezt = misc.tile([PC, PC], BF16, tag="m9")
                    nc.vector.tensor_copy(ezt, ezt_ps)
                    wsc = misc.tile([PC, C], BF16, tag="m10")
                    nc.vector.tensor_scalar_mul(wsc, (wp0, wp1)[P], recip)
                    pmp = ps.tile([PC, C], F32, tag="ps")
                    nc.tensor.matmul(pmp, ezt, wsc, start=True, stop=True)
                    # [128, 256] zero-padded so the fused output matmuls are
                    # full-array K=128 / M=128 (no row/col-group masks)
                    mp = stats.tile([128, 256], BF16, tag=f"mp{P}")
                    nc.vector.memset(mp, 0.0)
                    # descale the fp8e3m4-scaled v channels (partition dim = d)
                    nc.vector.tensor_scalar_mul(mp[0:PC, 0:C], pmp, (svinv0, svinv1)[P])
                    mps[P] = mp

                def emit_vtb0(nb):
                    ptv = pst.tile([PC, 512], BF16, tag="tp", name=f"ptv0_{nb}")
                    for hh in range(4):
                        nc.tensor.transpose(
                            ptv[:, hh * 128 : (hh + 1) * 128],
                            bv_hc[:, nb * 4 + hh, 0:PC],
                            identb,
                        )
                    nc.vector.tensor_copy(
                        vtb0[0:PC, nb * 512 : (nb + 1) * 512], ptv
                    )

                def v_extra(g):
                    # sprinkle QK^T matmuls through the dense v-dw stream
                    # (front-loaded: done before the vtb0 transposes pile on)
                    end = min(len(qk_jobs), (g + 1) * 11)
                    while qk_i[0] < end:
                        P, h = qk_jobs[qk_i[0]]
                        nc.tensor.matmul(
                            attnps[P],
                            bk_hc[:, h, PC * P : PC * P + PC],
                            bq_hc[:, h, PC * P : PC * P + PC],
                            start=(h == 0),
                            stop=(h == Himg - 1),
                        )
                        qk_i[0] += 1
                    if g == 24:
                        sm_state[0] = softmax_piece1(0)
                    elif g == 25:
                        softmax_piece2(0, *sm_state[0])
                        sm_state[1] = softmax_piece1(1)
                    elif g == 26:
                        softmax_piece2(1, *sm_state[1])
                    elif g == 27:
                        mps_prep(0)
                    elif g == 28:
                        mps_prep(1)
                    if g >= 24:
                        j = g - 24 + 1          # 1..24
                        target = j * NB0 // 24
                        while nb_done[0] < target:
                            emit_vtb0(nb_done[0])
                            nb_done[0] += 1

                # ---- E: v depthwise with everything interleaved ----
                dw_phase(bv3, tv_d, 7, F8E3, tpoolb, "twb", extra=v_extra)
                while nb_done[0] < NB0:
                    emit_vtb0(nb_done[0])
                    nb_done[0] += 1

                # ---- G tail: remaining transposes + fused (attn@v)+proj ----
                # pre-zero the pad rows of the scratch ring buffers once; the
                # in-loop copies only touch rows 0:PC, so the zeros persist
                for i in range(2):
                    z1 = scratch.tile([128, 512], BF16, tag="vtb1", name=f"z1_{i}")
                    nc.gpsimd.memset(z1[PC:128, :], 0.0)
                    z0 = scratch.tile([128, 512], BF16, tag="vtb0b", name=f"z0_{i}")
                    nc.gpsimd.memset(z0[PC:128, :], 0.0)
                for nb in range(Himg // 4):
                    h0 = nb * 4
                    ptv = pst.tile([PC, 512], BF16, tag="tp", name=f"ptv1_{nb}")
                    for hh in range(4):
                        nc.tensor.transpose(
                            ptv[:, hh * 128 : (hh + 1) * 128],
                            bv_hc[:, h0 + hh, PC : PC + PC],
                            identb,
                        )
                    vtb1 = scratch.tile([128, 512], BF16, tag="vtb1", name=f"vtb1_{nb}")
                    nc.vector.tensor_copy(vtb1[0:PC, :], ptv)
                    if nb >= NB0:
                        ptv0 = pst.tile([PC, 512], BF16, tag="tp", name=f"ptv0b_{nb}")
                        for hh in range(4):
                            nc.tensor.transpose(
                                ptv0[:, hh * 128 : (hh + 1) * 128],
                                bv_hc[:, h0 + hh, 0:PC],
                                identb,
                            )
                        vtb0b = scratch.tile([128, 512], BF16, tag="vtb0b", name=f"vtb0b_{nb}")
                        nc.scalar.activation(out=vtb0b[0:PC, :], in_=ptv0, func=AF.Copy)
                        vsrc0 = vtb0b
                    else:
                        vsrc0 = vtb0[:, nb * 512 : (nb + 1) * 512]
                    n = nb * 512
                    for mi, (r0, r1) in enumerate(((0, 128), (128, 192))):
                        mw = r1 - r0
                        po = ps.tile([128, 512], F32, tag="ps", name=f"po_{mi}_{nb}")
                        nc.tensor.matmul(
                            po, mps[0][:, mi * 128 : mi * 128 + 128], vsrc0,
                            start=True, stop=False,
                        )
                        nc.tensor.matmul(
                            po, mps[1][:, mi * 128 : mi * 128 + 128], vtb1,
                            start=False, stop=True,
                        )
                        so = ostage.tile([mw, 512], F32, tag="os", name=f"so_{mi}_{nb}")
                        copy_on(mi, so, po[0:mw, :])
                        nc.sync.dma_start(out=out_d[r0:r1, n : n + 512], in_=so)

    _split_excess_waits(nc)
    return nc


def _get_program():
    global _PROG
    if _PROG is None:
        _PROG = _build_program()
    return _PROG


def kernel(x, y, q_w, q_dw_w, kv_w, kv_dw_w, proj_w, temperature):
    return _run(x, y, q_w, q_dw_w, kv_w, kv_dw_w, proj_w, temperature)[0]


def _run(x, y, q_w, q_dw_w, kv_w, kv_dw_w, proj_w, temperature, trace=False):
    from concourse.bass_utils import run_bass_kernel_spmd

    x = np.asarray(x, dtype=np.float32).reshape(B, C, HW).astype(ml_dtypes.bfloat16)
    y = np.asarray(y, dtype=np.float32).reshape(B, C, HW).astype(ml_dtypes.bfloat16)
    q_w = np.asarray(q_w, dtype=np.float32)
    kv_w = np.asarray(kv_w, dtype=np.float32)
    proj_w = np.asarray(proj_w, dtype=np.float32)
    q_dw_w = np.asarray(q_dw_w, dtype=np.float32)
    kv_dw_w = np.asarray(kv_dw_w, dtype=np.float32)
    temperature = np.asarray(temperature, dtype=np.float32).reshape(HEADS)

    # conv weights padded to 256 K-rows (zeros) for K=128-homogeneous matmuls
    wq = np.zeros((256, C), np.float32)
    wq[0:C] = q_w[:, :, 0, 0].T
    wq = wq.astype(ml_dtypes.bfloat16)
    wkv = np.zeros((256, 384), np.float32)
    wkv[0:C] = kv_w[:, :, 0, 0].T
    wkv = wkv.astype(ml_dtypes.bfloat16)
    # v depthwise in fp8e3m4 with per-channel power-of-2 scaling; inverse
    # scales ride a [PC,1] per-partition multiply on the fused projection
    wv = kv_dw_w[C : 2 * C, 0]                              # [C, 7, 7]
    v_absmax = np.abs(wv).reshape(C, -1).max(axis=1)
    v_scale = 2.0 ** np.floor(np.log2(14.0 / v_absmax))
    svinv = (1.0 / v_scale).astype(np.float32).reshape(2, PC, 1)
    wpT = proj_w[:, :, 0, 0].T                              # [c_in, c_out]
    wp = np.stack([wpT[0:PC], wpT[PC:C]]).astype(ml_dtypes.bfloat16)
    tq = _build_toeplitz(q_dw_w[:, 0], 3, ml_dtypes.float8_e4m3)
    tk = _build_toeplitz(kv_dw_w[0:C, 0], 7, ml_dtypes.float8_e4m3)
    tv = _build_toeplitz(wv, 7, ml_dtypes.float8_e3m4, scales=v_scale)
    idb = np.eye(128, dtype=ml_dtypes.bfloat16)
    maskbd = np.zeros((PC, PC), np.float32)
    maskbd[0:DHC, 0:DHC] = 1.0
    maskbd[DHC:PC, DHC:PC] = 1.0
    temprow = np.repeat(temperature, DHC).reshape(1, C)

    shared = {
        "wq": wq, "wkv": wkv, "wp": wp, "tq": tq, "tk": tk, "tv": tv,
        "idb": idb, "maskbd": maskbd, "temprow": temprow, "svinv": svinv,
    }
    in_maps = [dict(shared, x=x[i], y=y[i]) for i in range(B)]

    nc = _get_program()
    res = run_bass_kernel_spmd(
        nc, in_maps, core_ids=list(range(B)), trace=trace
    )
    out = np.stack([res.results[i]["out"] for i in range(B)])
    return out.reshape(B, C, Himg, Wimg).astype(np.float32), res
